# revision 18
# baseline (speedup 1.0000x reference)
"""CvT self-attention (depthwise-conv QKV projection + MHA) on 8 Trainium2 cores.

Sharding: data-parallel over batch B=64 -> 8 batches per core. No collectives.

Per-core pipeline (per batch, all matmuls fp16 w/ fp32 PSUM accumulation):
  1. DMA hidden [1025, 384] fp32, convert fp16, PE-transpose to channel-major
     x_pad [c, 34, 34] (zero-padded spatially).
  2. Depthwise 3x3 conv + folded BN as 9 diagonal-matmul taps accumulating in
     PSUM (q: stride 1, k/v: stride 2 via strided access patterns).
  3. QKV linear projections. q/k produce [c_out, tokens]; v is computed with
     conv output as the stationary operand producing token-major [t, c_out].
  4. Attention per head, scores TRANSPOSED ([t, l]) so no transpose is needed
     between softmax and PV: scoresT = kh^T qh, exp (no max subtraction --
     scores are O(1)), PV with ones-augmented V so the softmax denominator
     falls out of the same matmul, then PE-transpose [65, l] -> [l, 65] and
     normalize by the denominator column.
"""

import sys

sys.path.insert(0, "/opt/trn_rl_repo")

import numpy as np

import concourse.bass as bass
import concourse.mybir as mybir
import concourse.tile as tile
from concourse.masks import make_identity
from concourse.vector_clock import ScopedClock

B, C, H, W = 64, 384, 32, 32
NH, HD = 6, 64
L = 1 + H * W  # 1025 query tokens
TK = 1 + (H // 2) * (W // 2)  # 257 key/value tokens
NCORES = 8
BPC = B // NCORES  # batches per core
EPS = 1e-5
F16 = mybir.dt.float16
F32 = mybir.dt.float32
Act = mybir.ActivationFunctionType

TRACE = False
LAST_EXEC_NS = None

# l chunks for the 1025-token free dim (balanced, PSUM bank = 512 fp32)
LCH = [(0, 342), (342, 342), (684, 341)]
# t chunks for the 257-token key dim over partitions
TCH = [(0, 128), (128, 128), (256, 1)]


def _split_multi_waits(nc):
    """walrus in this image only allows ONE sync wait per instruction. Move
    extra waits onto NoOps (same engine) inserted just before the offender."""
    from bass_rust import InstNoOp

    n_split = 0
    for blk in nc.m.functions[0].blocks:
        insts = blk.instructions
        out_list = []
        changed = False
        for inst in insts:
            si = inst.sync_info
            waits = list(si.on_wait) if si and si.on_wait else []
            if len(waits) > 1:
                changed = True
                for w in waits[:-1]:
                    n_split += 1
                    nop = InstNoOp(name=f"I-waitsplit-{n_split}", ins=[], outs=[])
                    nop.engine = inst.engine
                    nop.sync_info = mybir.SyncInfo(on_wait=[w], on_update=[])
                    out_list.append(nop)
                si.on_wait = waits[-1:]
            out_list.append(inst)
        if changed:
            blk.instructions = out_list


def _dedup_ldweights(nc):
    """Drop an LDWEIGHTS when the PE already has exactly those weights loaded
    (previous LDW had an identical access pattern and no other PE instruction
    invalidated them). Waits carried by a dropped LDW move to the next PE
    instruction. Safe here because every tile tensor is written before its
    first LDW and never rewritten."""
    removed = 0
    for blk in nc.m.functions[0].blocks:
        insts = blk.instructions
        out_list = []
        last_key = None
        pending = []
        for inst in insts:
            tn = type(inst).__name__
            eng = inst.engine
            if tn == "InstLdweights":
                key = (
                    inst.ins[0].concise(),
                    str(inst.tile_position),
                    str(inst.tile_size),
                    bool(inst.is_transpose),
                    str(inst.perf_mode),
                )
                if key == last_key:
                    si = inst.sync_info
                    if si and si.on_wait:
                        pending.extend(list(si.on_wait))
                    removed += 1
                    continue
                last_key = key
            elif tn != "InstMatmult" and eng == mybir.EngineType.PE:
                last_key = None
            if pending and eng == mybir.EngineType.PE:
                si = inst.sync_info
                if si is None:
                    inst.sync_info = mybir.SyncInfo(on_wait=pending, on_update=[])
                else:
                    si.on_wait = list(si.on_wait or []) + pending
                pending = []
            out_list.append(inst)
        blk.instructions = out_list
    return removed


def _patch_drain():
    """Append wait-splitting to the end of TileContext's tail drain."""
    if getattr(tile.TileContext, "_drain_patched", False):
        return

    def _drain_and_barrier(self, tick_clock, wait_clock):
        nc = self.nc
        drain_inst = nc.sync.drain()
        wait_clock.add_sem_waits(
            drain_inst.ins, ScopedClock({None: tick_clock.global_clock})
        )
        nc.all_engine_barrier()
        assert self.sems is not None
        popped = nc._tile_sem_poison_stack.pop()
        assert popped is self._sem_poison
        nc.clear_and_free_semaphores(list(self.sems.allocated().values()))
        nc.all_engine_barrier()
        _dedup_ldweights(nc)
        _split_multi_waits(nc)

    tile.TileContext._drain_and_barrier = _drain_and_barrier
    tile.TileContext._drain_patched = True


def _build_kernel():
    _patch_drain()
    nc = bass.Bass()
    hid = nc.dram_tensor("hid", [BPC, L, C], F32, kind="ExternalInput").ap()
    wdiag = nc.dram_tensor("wdiag", [128, 81, 128], F16, kind="ExternalInput").ap()
    wproj = nc.dram_tensor("wproj", [128, 18, 128], F16, kind="ExternalInput").ap()
    wpv = nc.dram_tensor("wpv", [128, 3, 384], F16, kind="ExternalInput").ap()
    biases = nc.dram_tensor("biases", [128, 15], F32, kind="ExternalInput").ap()
    out = nc.dram_tensor("out", [BPC, L, C], F32, kind="ExternalOutput").ap()

    with tile.TileContext(nc) as tc:
        with (
            tc.tile_pool(name="const", bufs=1) as const,
            tc.tile_pool(name="io", bufs=3) as io,
            tc.tile_pool(name="stage", bufs=2) as stage,
            tc.tile_pool(name="ctx", bufs=8) as ctxp,
            tc.tile_pool(name="outp", bufs=3) as outp,
            tc.tile_pool(name="small", bufs=4) as small,
            tc.tile_pool(name="pmm", bufs=3, space="PSUM") as pmm,
            tc.tile_pool(name="pctx", bufs=3, space="PSUM") as pctx,
            tc.tile_pool(name="ptp", bufs=2, space="PSUM") as ptp,
        ):
            # ---- constants ----
            wd_sb = const.tile([128, 81, 128], F16, tag="wd")
            nc.sync.dma_start(out=wd_sb[:], in_=wdiag)
            wp_sb = const.tile([128, 18, 128], F16, tag="wp")
            nc.sync.dma_start(out=wp_sb[:], in_=wproj)
            wpv_sb = const.tile([128, 3, 384], F16, tag="wpv")
            nc.sync.dma_start(out=wpv_sb[:], in_=wpv)
            bias_sb = const.tile([128, 15], F32, tag="bias")
            nc.sync.dma_start(out=bias_sb[:], in_=biases)
            ident = const.tile([128, 128], F16, tag="ident")
            make_identity(nc, ident[:])

            for b in range(BPC):
                # ---- stage A: load (casting DMA) + DMA-transpose to channel-major ----
                x_pad = stage.tile([128, 3, 34, 34], F16, tag="xpad")
                # zero the 1-px border (interior is fully overwritten)
                nc.gpsimd.memset(x_pad[:, :, 0, :], 0.0)
                nc.gpsimd.memset(x_pad[:, :, 33, :], 0.0)
                nc.gpsimd.memset(x_pad[:, :, 1:33, 0], 0.0)
                nc.gpsimd.memset(x_pad[:, :, 1:33, 33], 0.0)

                for k in range(8):
                    x16 = io.tile([128, 384], F16, tag="x16")
                    nc.gpsimd.dma_start(
                        out=x16[:], in_=hid[b, 1 + 128 * k : 1 + 128 * (k + 1), :]
                    )
                    for cc in range(3):
                        tp = ptp.tile([128, 128], F16, tag="tp")
                        nc.tensor.transpose(
                            tp[:], x16[:, cc * 128 : (cc + 1) * 128], ident[:]
                        )
                        # tokens 128k..128k+127 = image rows 4k..4k+3
                        nc.vector.tensor_copy(
                            x_pad[:, cc, 1 + 4 * k : 5 + 4 * k, 1:33], tp[:]
                        )

                cls16 = small.tile([128, 3], F16, tag="cls")
                for cc in range(3):
                    nc.gpsimd.dma_start(
                        out=cls16[:, cc : cc + 1],
                        in_=hid[b, 0:1, cc * 128 : (cc + 1) * 128].rearrange(
                            "a b -> b a"
                        ),
                    )

                # ---- stage B: depthwise conv + BN (diagonal matmuls) ----
                q_src = stage.tile([128, 3, 1025], F16, tag="qsrc")
                k_src = stage.tile([128, 3, 257], F16, tag="ksrc")
                v_src = stage.tile([128, 3, 257], F16, tag="vsrc")
                for cc in range(3):
                    for s in (q_src, k_src, v_src):
                        nc.gpsimd.tensor_copy(s[:, cc, 0:1], cls16[:, cc : cc + 1])
                for cc in range(3):
                    # q: stride 1, two 512-token banks; taps outer so both
                    # banks' matmuls share one LDWEIGHTS
                    psq = [pmm.tile([128, 512], F32, tag="mm", name=f"psq{b}_{cc}_{i}") for i in range(2)]
                    for tap in range(9):
                        di, dj = tap // 3, tap % 3
                        with tc.tile_critical():
                            for nb in range(2):
                                rhs = x_pad[
                                    :,
                                    cc,
                                    16 * nb + di : 16 * nb + di + 16,
                                    dj : dj + 32,
                                ]
                                nc.tensor.matmul(
                                    psq[nb][:],
                                    wd_sb[:, tap * 3 + cc, :],
                                    rhs,
                                    start=(tap == 0),
                                    stop=(tap == 8),
                                )
                    for nb in range(2):
                        nc.scalar.activation(
                            q_src[:, cc, 1 + 512 * nb : 513 + 512 * nb],
                            psq[nb][:],
                            Act.Identity,
                            bias=bias_sb[:, cc : cc + 1],
                        )
                    # k, v: stride 2 (16x16 outputs)
                    xv = x_pad[:, cc].rearrange(
                        "p (i ti) (j tj) -> p i ti j tj", ti=2, tj=2
                    )
                    for ci, src in ((1, k_src), (2, v_src)):
                        ps = pmm.tile([128, 512], F32, tag="mm")
                        for tap in range(9):
                            di, dj = tap // 3, tap % 3
                            rhs = xv[
                                :,
                                di // 2 : di // 2 + 16,
                                di % 2,
                                dj // 2 : dj // 2 + 16,
                                dj % 2,
                            ]
                            nc.tensor.matmul(
                                ps[:, :256],
                                wd_sb[:, ci * 27 + tap * 3 + cc, :],
                                rhs,
                                start=(tap == 0),
                                stop=(tap == 8),
                            )
                        nc.scalar.activation(
                            src[:, cc, 1:257],
                            ps[:, :256],
                            Act.Identity,
                            bias=bias_sb[:, ci * 3 + cc : ci * 3 + cc + 1],
                        )

                # ---- stage C: projections (kc inner-adjacent for LDW reuse) ----
                qh = stage.tile([128, 3, 1025], F16, tag="qh")
                kh = stage.tile([128, 3, 257], F16, tag="kh")
                for mc in range(3):
                    ps3 = [pmm.tile([128, 512], F32, tag="mm", name=f"ps3_{b}_{mc}_{i}") for i in range(3)]
                    for kc in range(3):
                        with tc.tile_critical():
                            for nci, (n0, nl) in enumerate(LCH):
                                nc.tensor.matmul(
                                    ps3[nci][:, :nl],
                                    wp_sb[:, kc * 3 + mc, :],
                                    q_src[:, kc, n0 : n0 + nl],
                                    start=(kc == 0),
                                    stop=(kc == 2),
                                )
                    for nci, (n0, nl) in enumerate(LCH):
                        nc.vector.tensor_scalar_add(
                            qh[:, mc, n0 : n0 + nl],
                            ps3[nci][:, :nl],
                            bias_sb[:, 9 + mc : 10 + mc],
                        )
                    ps = pmm.tile([128, 512], F32, tag="mm")
                    for kc in range(3):
                        nc.tensor.matmul(
                            ps[:, :257],
                            wp_sb[:, 9 + kc * 3 + mc, :],
                            k_src[:, kc, :],
                            start=(kc == 0),
                            stop=(kc == 2),
                        )
                    nc.vector.tensor_scalar_add(
                        kh[:, mc, :], ps[:, :257], bias_sb[:, 12 + mc : 13 + mc]
                    )
                v_store = stage.tile([128, 3, 6, 65], F16, tag="vst")
                nc.gpsimd.memset(v_store[:, :, :, 64:65], 1.0)
                for tcc, (t0, tl) in enumerate(TCH):
                    ps = pmm.tile([128, 512], F32, tag="mm")
                    for kc in range(3):
                        nc.tensor.matmul(
                            ps[:tl, :384],
                            v_src[:, kc, t0 : t0 + tl],
                            wpv_sb[:, kc, :],
                            start=(kc == 0),
                            stop=(kc == 2),
                        )
                    nc.vector.tensor_copy(
                        v_store[:tl, tcc, :, 0:64],
                        ps[:tl, :384].rearrange("p (h d) -> p h d", h=6),
                    )

                # ---- stage D/E: attention per head ----
                ctx_tiles = []
                for h in range(6):
                    base, ch = (h % 2) * 64, h // 2
                    expT = stage.tile([128, 3, 1025], F16, tag="expT", bufs=3)
                    for tcc, (t0, tl) in enumerate(TCH):
                        ps3s = [
                            pmm.tile([128, 512], F32, tag="mm", name=f"sc{b}_{h}_{tcc}_{i}")
                            for i in range(3)
                        ]
                        with tc.tile_critical():
                            for nci, (n0, nl) in enumerate(LCH):
                                nc.tensor.matmul(
                                    ps3s[nci][:tl, :nl],
                                    kh[base : base + 64, ch, t0 : t0 + tl],
                                    qh[base : base + 64, ch, n0 : n0 + nl],
                                    start=True,
                                    stop=True,
                                )
                        for nci, (n0, nl) in enumerate(LCH):
                            nc.scalar.activation(
                                expT[:tl, tcc, n0 : n0 + nl],
                                ps3s[nci][:tl, :nl],
                                Act.Exp,
                            )
                    # PV: tc outer so the 3 l-chunks share each LDWEIGHTS
                    ctxsb = ctxp.tile([128, 1152], F16, tag="ctx")
                    ctx_tiles.append(ctxsb)
                    cps3 = [pctx.tile([65, 512], F32, tag="pc", name=f"cps{b}_{h}_{i}") for i in range(3)]
                    for tcc, (t0, tl) in enumerate(TCH):
                        with tc.tile_critical():
                            for nci, (n0, nl) in enumerate(LCH):
                                nc.tensor.matmul(
                                    cps3[nci][:, :nl],
                                    v_store[:tl, tcc, h, :],
                                    expT[:tl, tcc, n0 : n0 + nl],
                                    start=(tcc == 0),
                                    stop=(tcc == 2),
                                )
                    for nci, (n0, nl) in enumerate(LCH):
                        nc.scalar.activation(
                            ctxsb[:65, n0 : n0 + nl], cps3[nci][:, :nl], Act.Copy
                        )

                # ---- stage F: DMA-transpose back, normalize, store ----
                for lc in range(9):
                    l0 = lc * 128
                    ll = min(128, L - l0)
                    osb = outp.tile([128, 384], F32, tag="osb")
                    for h in range(6):
                        tp = ptp.tile([128, 65], F16, tag="tp")
                        nc.tensor.transpose(
                            tp[:ll, :],
                            ctx_tiles[h][:65, l0 : l0 + ll],
                            ident[:65, :65],
                        )
                        rec = small.tile([128, 1], F32, tag="rec")
                        nc.vector.reciprocal(rec[:ll], tp[:ll, 64:65])
                        nc.vector.tensor_scalar_mul(
                            osb[:ll, h * 64 : (h + 1) * 64], tp[:ll, 0:64], rec[:ll]
                        )
                    nc.sync.dma_start(
                        out=out[b, l0 : l0 + ll, :], in_=osb[:ll, :]
                    )
    return nc


def _install_trace_support():
    """Provide the NTFF profile hook (this image's antenv lacks axon_hooks)
    and neuter the artifact upload (no fish access here)."""
    import contextlib
    import ctypes
    import types

    import concourse.bass_utils as bu

    bu.upload_artifacts = lambda tmpdir: f"local:{tmpdir}"
    try:
        from antenv.axon_hooks import get_axon_ntff_profile_hook  # noqa: F401

        return
    except ImportError:
        pass
    so_path = "/opt/axon/libaxon_pjrt.so"
    lib = ctypes.CDLL(so_path)
    if not hasattr(lib, "axon_start_nrt_profile"):
        return
    lib.axon_start_nrt_profile.argtypes = [
        ctypes.POINTER(ctypes.c_int64),
        ctypes.c_size_t,
    ]
    lib.axon_start_nrt_profile.restype = ctypes.c_int64
    lib.axon_stop_nrt_profile.argtypes = [ctypes.c_char_p]
    lib.axon_stop_nrt_profile.restype = ctypes.c_int64

    @contextlib.contextmanager
    def _hook(output_dir, device_ids):
        import jax

        jax.devices()
        if device_ids:
            ids = (ctypes.c_int64 * len(device_ids))(*device_ids)
            rc = lib.axon_start_nrt_profile(ids, len(device_ids))
        else:
            rc = lib.axon_start_nrt_profile(None, 0)
        if rc != 0:
            raise RuntimeError(f"axon_start_nrt_profile rc={rc}")
        try:
            yield
        finally:
            n = lib.axon_stop_nrt_profile(str(output_dir).encode())
            print(f"profile: {n} file(s) written to {output_dir}")

    import antenv

    mod = types.ModuleType("antenv.axon_hooks")
    holder = {"h": _hook}
    mod.get_axon_ntff_profile_hook = lambda: holder["h"]
    mod.set_axon_ntff_profile_hook = lambda h: holder.__setitem__("h", h)
    antenv.axon_hooks = mod
    sys.modules["antenv.axon_hooks"] = mod


_CACHED = None


def _prep_weights(inputs):
    """Fold BN into conv weights; pre-transpose/chunk projection weights."""
    f16 = np.float16
    wdiag = np.zeros((128, 81, 128), dtype=f16)
    biases = np.zeros((128, 15), dtype=np.float32)
    wproj = np.zeros((128, 18, 128), dtype=f16)
    wpv = np.zeros((128, 3, 384), dtype=f16)
    for ci, p in enumerate(["q", "k", "v"]):
        gamma = np.asarray(inputs[f"bn_{p}_gamma"], np.float64)
        var = np.asarray(inputs[f"bn_{p}_var"], np.float64)
        beta = np.asarray(inputs[f"bn_{p}_beta"], np.float64)
        mean = np.asarray(inputs[f"bn_{p}_mean"], np.float64)
        inv = gamma / np.sqrt(var + EPS)
        wfold = np.asarray(inputs[f"conv_{p}_w"], np.float64)[:, 0] * inv[:, None, None]
        bias_c = beta - mean * inv
        for tap in range(9):
            di, dj = tap // 3, tap % 3
            for cc in range(3):
                d = wfold[cc * 128 : (cc + 1) * 128, di, dj]
                np.fill_diagonal(wdiag[:, ci * 27 + tap * 3 + cc, :], d.astype(f16))
        for cc in range(3):
            biases[:, ci * 3 + cc] = bias_c[cc * 128 : (cc + 1) * 128]
        w = np.asarray(inputs[f"w_{p}"], np.float64)  # [o, c]
        assert np.abs(np.asarray(inputs[f"b_{p}"])).max() == 0.0 or p != "v", (
            "nonzero v bias unsupported"
        )
        if p == "q":
            wt = (w.T * (C**-0.5)).astype(f16)  # fold attention scale
        else:
            wt = w.T.astype(f16)
        if p in ("q", "k"):
            pi = 0 if p == "q" else 1
            for kc in range(3):
                for mc in range(3):
                    wproj[:, pi * 9 + kc * 3 + mc, :] = wt[
                        kc * 128 : (kc + 1) * 128, mc * 128 : (mc + 1) * 128
                    ]
            # projection bias (spec: zeros, but supported per out-channel)
            bvec = np.asarray(inputs[f"b_{p}"], np.float64) * (
                (C**-0.5) if p == "q" else 1.0
            )
            for mc in range(3):
                biases[:, 9 + pi * 3 + mc] = bvec[mc * 128 : (mc + 1) * 128]
        else:
            for kc in range(3):
                wpv[:, kc, :] = wt[kc * 128 : (kc + 1) * 128, :]
    return wdiag, wproj, wpv, biases


def kernel(**inputs) -> np.ndarray:
    global _CACHED, LAST_EXEC_NS
    from concourse.bass_utils import run_bass_kernel_spmd

    if TRACE:
        _install_trace_support()
    hidden = np.ascontiguousarray(np.asarray(inputs["hidden_state"], np.float32))
    assert hidden.shape == (B, L, C)
    wdiag, wproj, wpv, biases = _prep_weights(inputs)

    if _CACHED is None:
        _CACHED = _build_kernel()
    nc = _CACHED

    in_maps = []
    for core in range(NCORES):
        in_maps.append(
            {
                "hid": hidden[core * BPC : (core + 1) * BPC],
                "wdiag": wdiag,
                "wproj": wproj,
                "wpv": wpv,
                "biases": biases,
            }
        )
    res = run_bass_kernel_spmd(
        nc, in_maps, core_ids=list(range(NCORES)), trace=TRACE
    )
    LAST_EXEC_NS = res.exec_time_ns
    out = np.concatenate([res.results[i]["out"] for i in range(NCORES)], axis=0)
    return out.astype(np.float32)


# revision 20
# speedup vs baseline: 3.6354x; 3.6354x over previous
"""CvT self-attention (depthwise-conv QKV projection + MHA) on 8 Trainium2 cores.

Sharding: data-parallel over batch B=64 -> 8 batches per core. No collectives.

Per-core pipeline (per batch, all matmuls fp16 w/ fp32 PSUM accumulation):
  1. DMA hidden [1025, 384] fp32, convert fp16, PE-transpose to channel-major
     x_pad [c, 34, 34] (zero-padded spatially).
  2. Depthwise 3x3 conv + folded BN as 9 diagonal-matmul taps accumulating in
     PSUM (q: stride 1, k/v: stride 2 via strided access patterns).
  3. QKV linear projections. q/k produce [c_out, tokens]; v is computed with
     conv output as the stationary operand producing token-major [t, c_out].
  4. Attention per head, scores TRANSPOSED ([t, l]) so no transpose is needed
     between softmax and PV: scoresT = kh^T qh, exp (no max subtraction --
     scores are O(1)), PV with ones-augmented V so the softmax denominator
     falls out of the same matmul, then PE-transpose [65, l] -> [l, 65] and
     normalize by the denominator column.
"""

import sys

sys.path.insert(0, "/opt/trn_rl_repo")

import numpy as np

import concourse.bass as bass
import concourse.mybir as mybir
import concourse.tile as tile
from concourse.masks import make_identity
from concourse.vector_clock import ScopedClock

B, C, H, W = 64, 384, 32, 32
NH, HD = 6, 64
L = 1 + H * W  # 1025 query tokens
TK = 1 + (H // 2) * (W // 2)  # 257 key/value tokens
NCORES = 8
BPC = B // NCORES  # batches per core
EPS = 1e-5
F16 = mybir.dt.float16
F32 = mybir.dt.float32
Act = mybir.ActivationFunctionType

TRACE = False
LAST_EXEC_NS = None

# l chunks for the 1025-token free dim (balanced, PSUM bank = 512 fp32)
LCH = [(0, 342), (342, 342), (684, 341)]
# t chunks for the 257-token key dim over partitions
TCH = [(0, 128), (128, 128), (256, 1)]


def _split_multi_waits(nc):
    """walrus in this image only allows ONE sync wait per instruction. Move
    extra waits onto NoOps (same engine) inserted just before the offender."""
    from bass_rust import InstNoOp

    n_split = 0
    for blk in nc.m.functions[0].blocks:
        insts = blk.instructions
        out_list = []
        changed = False
        for inst in insts:
            si = inst.sync_info
            waits = list(si.on_wait) if si and si.on_wait else []
            if len(waits) > 1:
                changed = True
                for w in waits[:-1]:
                    n_split += 1
                    nop = InstNoOp(name=f"I-waitsplit-{n_split}", ins=[], outs=[])
                    nop.engine = inst.engine
                    nop.sync_info = mybir.SyncInfo(on_wait=[w], on_update=[])
                    out_list.append(nop)
                si.on_wait = waits[-1:]
            out_list.append(inst)
        if changed:
            blk.instructions = out_list


def _dedup_ldweights(nc):
    """Drop an LDWEIGHTS when the PE already has exactly those weights loaded
    (previous LDW had an identical access pattern and no other PE instruction
    invalidated them). Waits carried by a dropped LDW move to the next PE
    instruction. Safe here because every tile tensor is written before its
    first LDW and never rewritten."""
    removed = 0
    for blk in nc.m.functions[0].blocks:
        insts = blk.instructions
        out_list = []
        last_key = None
        pending = []
        for inst in insts:
            tn = type(inst).__name__
            eng = inst.engine
            if tn == "InstLdweights":
                key = (
                    inst.ins[0].concise(),
                    str(inst.tile_position),
                    str(inst.tile_size),
                    bool(inst.is_transpose),
                    str(inst.perf_mode),
                )
                if key == last_key:
                    si = inst.sync_info
                    if si and si.on_wait:
                        pending.extend(list(si.on_wait))
                    removed += 1
                    continue
                last_key = key
            elif tn != "InstMatmult" and eng == mybir.EngineType.PE:
                last_key = None
            if pending and eng == mybir.EngineType.PE:
                si = inst.sync_info
                if si is None:
                    inst.sync_info = mybir.SyncInfo(on_wait=pending, on_update=[])
                else:
                    si.on_wait = list(si.on_wait or []) + pending
                pending = []
            out_list.append(inst)
        blk.instructions = out_list
    return removed


def _patch_drain():
    """Append wait-splitting to the end of TileContext's tail drain."""
    if getattr(tile.TileContext, "_drain_patched", False):
        return

    def _drain_and_barrier(self, tick_clock, wait_clock):
        nc = self.nc
        drain_inst = nc.sync.drain()
        wait_clock.add_sem_waits(
            drain_inst.ins, ScopedClock({None: tick_clock.global_clock})
        )
        nc.all_engine_barrier()
        assert self.sems is not None
        popped = nc._tile_sem_poison_stack.pop()
        assert popped is self._sem_poison
        nc.clear_and_free_semaphores(list(self.sems.allocated().values()))
        nc.all_engine_barrier()
        _dedup_ldweights(nc)
        _split_multi_waits(nc)

    tile.TileContext._drain_and_barrier = _drain_and_barrier
    tile.TileContext._drain_patched = True


def _build_kernel():
    _patch_drain()
    nc = bass.Bass()
    hid = nc.dram_tensor("hid", [BPC, L, C], F32, kind="ExternalInput").ap()
    wdiag = nc.dram_tensor("wdiag", [128, 81, 128], F16, kind="ExternalInput").ap()
    wproj = nc.dram_tensor("wproj", [128, 18, 128], F16, kind="ExternalInput").ap()
    wpv = nc.dram_tensor("wpv", [128, 3, 384], F16, kind="ExternalInput").ap()
    biases = nc.dram_tensor("biases", [128, 15], F32, kind="ExternalInput").ap()
    out = nc.dram_tensor("out", [BPC, L, C], F32, kind="ExternalOutput").ap()

    with tile.TileContext(nc) as tc:
        with (
            tc.tile_pool(name="const", bufs=1) as const,
            tc.tile_pool(name="io", bufs=3) as io,
            tc.tile_pool(name="stage", bufs=2) as stage,
            tc.tile_pool(name="ctx", bufs=8) as ctxp,
            tc.tile_pool(name="outp", bufs=3) as outp,
            tc.tile_pool(name="small", bufs=4) as small,
            tc.tile_pool(name="pmm", bufs=3, space="PSUM") as pmm,
            tc.tile_pool(name="pctx", bufs=3, space="PSUM") as pctx,
            tc.tile_pool(name="ptp", bufs=2, space="PSUM") as ptp,
        ):
            # ---- constants ----
            wd_sb = const.tile([128, 81, 128], F16, tag="wd")
            nc.sync.dma_start(out=wd_sb[:], in_=wdiag)
            wp_sb = const.tile([128, 18, 128], F16, tag="wp")
            nc.sync.dma_start(out=wp_sb[:], in_=wproj)
            wpv_sb = const.tile([128, 3, 384], F16, tag="wpv")
            nc.sync.dma_start(out=wpv_sb[:], in_=wpv)
            bias_sb = const.tile([128, 15], F32, tag="bias")
            nc.sync.dma_start(out=bias_sb[:], in_=biases)
            ident = const.tile([128, 128], F16, tag="ident")
            make_identity(nc, ident[:])

            def emit_load(b):
                # ---- stage A: load (casting DMA) + PE-transpose to channel-major ----
                x_pad = stage.tile([128, 3, 34, 34], F16, tag="xpad", name=f"xpad{b}")
                # zero the 1-px border (interior is fully overwritten)
                nc.gpsimd.memset(x_pad[:, :, 0, :], 0.0)
                nc.gpsimd.memset(x_pad[:, :, 33, :], 0.0)
                nc.gpsimd.memset(x_pad[:, :, 1:33, 0], 0.0)
                nc.gpsimd.memset(x_pad[:, :, 1:33, 33], 0.0)

                for k in range(8):
                    x16 = io.tile([128, 384], F16, tag="x16", name=f"x16_{b}_{k}")
                    nc.gpsimd.dma_start(
                        out=x16[:], in_=hid[b, 1 + 128 * k : 1 + 128 * (k + 1), :]
                    )
                    for cc in range(3):
                        tp = ptp.tile([128, 128], F16, tag="tp", name=f"tpi{b}_{k}_{cc}")
                        nc.tensor.transpose(
                            tp[:], x16[:, cc * 128 : (cc + 1) * 128], ident[:]
                        )
                        # tokens 128k..128k+127 = image rows 4k..4k+3
                        nc.vector.tensor_copy(
                            x_pad[:, cc, 1 + 4 * k : 5 + 4 * k, 1:33], tp[:]
                        )

                cls16 = small.tile([128, 3], F16, tag="cls", name=f"cls{b}")
                for cc in range(3):
                    nc.gpsimd.dma_start(
                        out=cls16[:, cc : cc + 1],
                        in_=hid[b, 0:1, cc * 128 : (cc + 1) * 128].rearrange(
                            "a b -> b a"
                        ),
                    )
                return {"x_pad": x_pad, "cls16": cls16}

            def emit_convproj(b, st):
                x_pad, cls16 = st["x_pad"], st["cls16"]
                # ---- stage B: depthwise conv + BN (diagonal matmuls) ----
                q_src = stage.tile([128, 3, 1025], F16, tag="qsrc", name=f"qsrc{b}")
                k_src = stage.tile([128, 3, 257], F16, tag="ksrc", name=f"ksrc{b}")
                v_src = stage.tile([128, 3, 257], F16, tag="vsrc", name=f"vsrc{b}")
                for cc in range(3):
                    for s in (q_src, k_src, v_src):
                        nc.gpsimd.tensor_copy(s[:, cc, 0:1], cls16[:, cc : cc + 1])
                for cc in range(3):
                    # q: stride 1, two 512-token banks; taps outer so both
                    # banks' matmuls share one LDWEIGHTS
                    psq = [
                        pmm.tile([128, 512], F32, tag="mm", name=f"psq{b}_{cc}_{i}")
                        for i in range(2)
                    ]
                    for tap in range(9):
                        di, dj = tap // 3, tap % 3
                        for nb in range(2):
                            rhs = x_pad[
                                :, cc, 16 * nb + di : 16 * nb + di + 16, dj : dj + 32
                            ]
                            nc.tensor.matmul(
                                psq[nb][:],
                                wd_sb[:, tap * 3 + cc, :],
                                rhs,
                                start=(tap == 0),
                                stop=(tap == 8),
                            )
                    for nb in range(2):
                        nc.scalar.activation(
                            q_src[:, cc, 1 + 512 * nb : 513 + 512 * nb],
                            psq[nb][:],
                            Act.Identity,
                            bias=bias_sb[:, cc : cc + 1],
                        )
                    # k, v: stride 2 (16x16 outputs)
                    xv = x_pad[:, cc].rearrange(
                        "p (i ti) (j tj) -> p i ti j tj", ti=2, tj=2
                    )
                    for ci, src in ((1, k_src), (2, v_src)):
                        ps = pmm.tile([128, 512], F32, tag="mm", name=f"pkv{b}_{cc}_{ci}")
                        for tap in range(9):
                            di, dj = tap // 3, tap % 3
                            rhs = xv[
                                :,
                                di // 2 : di // 2 + 16,
                                di % 2,
                                dj // 2 : dj // 2 + 16,
                                dj % 2,
                            ]
                            nc.tensor.matmul(
                                ps[:, :256],
                                wd_sb[:, ci * 27 + tap * 3 + cc, :],
                                rhs,
                                start=(tap == 0),
                                stop=(tap == 8),
                            )
                        nc.scalar.activation(
                            src[:, cc, 1:257],
                            ps[:, :256],
                            Act.Identity,
                            bias=bias_sb[:, ci * 3 + cc : ci * 3 + cc + 1],
                        )

                # ---- stage C: projections (kc inner-adjacent for LDW reuse) ----
                qh = stage.tile([128, 3, 1025], F16, tag="qh", name=f"qh{b}")
                kh = stage.tile([128, 3, 257], F16, tag="kh", name=f"kh{b}")
                for mc in range(3):
                    ps3 = [
                        pmm.tile([128, 512], F32, tag="mm", name=f"ps3_{b}_{mc}_{i}")
                        for i in range(3)
                    ]
                    for kc in range(3):
                        for nci, (n0, nl) in enumerate(LCH):
                            nc.tensor.matmul(
                                ps3[nci][:, :nl],
                                wp_sb[:, kc * 3 + mc, :],
                                q_src[:, kc, n0 : n0 + nl],
                                start=(kc == 0),
                                stop=(kc == 2),
                            )
                    for nci, (n0, nl) in enumerate(LCH):
                        nc.vector.tensor_scalar_add(
                            qh[:, mc, n0 : n0 + nl],
                            ps3[nci][:, :nl],
                            bias_sb[:, 9 + mc : 10 + mc],
                        )
                    ps = pmm.tile([128, 512], F32, tag="mm", name=f"pk{b}_{mc}")
                    for kc in range(3):
                        nc.tensor.matmul(
                            ps[:, :257],
                            wp_sb[:, 9 + kc * 3 + mc, :],
                            k_src[:, kc, :],
                            start=(kc == 0),
                            stop=(kc == 2),
                        )
                    nc.vector.tensor_scalar_add(
                        kh[:, mc, :], ps[:, :257], bias_sb[:, 12 + mc : 13 + mc]
                    )
                v_store = stage.tile([128, 3, 6, 65], F16, tag="vst", name=f"vst{b}")
                nc.gpsimd.memset(v_store[:, :, :, 64:65], 1.0)
                for tcc, (t0, tl) in enumerate(TCH):
                    ps = pmm.tile([128, 512], F32, tag="mm", name=f"pv{b}_{tcc}")
                    for kc in range(3):
                        nc.tensor.matmul(
                            ps[:tl, :384],
                            v_src[:, kc, t0 : t0 + tl],
                            wpv_sb[:, kc, :],
                            start=(kc == 0),
                            stop=(kc == 2),
                        )
                    nc.vector.tensor_copy(
                        v_store[:tl, tcc, :, 0:64],
                        ps[:tl, :384].rearrange("p (h d) -> p h d", h=6),
                    )
                st.update(qh=qh, kh=kh, v_store=v_store)

            def emit_attn(b, st):
                qh, kh, v_store = st["qh"], st["kh"], st["v_store"]
                # ---- stage D/E: attention per head ----
                ctx_tiles = []
                for h in range(6):
                    base, ch = (h % 2) * 64, h // 2
                    expT = stage.tile(
                        [128, 3, 1025], F16, tag="expT", bufs=3, name=f"expT{b}_{h}"
                    )
                    for tcc, (t0, tl) in enumerate(TCH):
                        for n0, nl in LCH:
                            ps = pmm.tile(
                                [128, 512], F32, tag="mm", name=f"sc{b}_{h}_{tcc}_{n0}"
                            )
                            nc.tensor.matmul(
                                ps[:tl, :nl],
                                kh[base : base + 64, ch, t0 : t0 + tl],
                                qh[base : base + 64, ch, n0 : n0 + nl],
                                start=True,
                                stop=True,
                            )
                            nc.scalar.activation(
                                expT[:tl, tcc, n0 : n0 + nl], ps[:tl, :nl], Act.Exp
                            )
                    # PV: tc outer so the 3 l-chunks share each LDWEIGHTS
                    ctxsb = ctxp.tile([128, 1152], F16, tag="ctx", name=f"ctx{b}_{h}")
                    ctx_tiles.append(ctxsb)
                    cps3 = [
                        pctx.tile([65, 512], F32, tag="pc", name=f"cps{b}_{h}_{i}")
                        for i in range(3)
                    ]
                    for tcc, (t0, tl) in enumerate(TCH):
                        for nci, (n0, nl) in enumerate(LCH):
                            nc.tensor.matmul(
                                cps3[nci][:, :nl],
                                v_store[:tl, tcc, h, :],
                                expT[:tl, tcc, n0 : n0 + nl],
                                start=(tcc == 0),
                                stop=(tcc == 2),
                            )
                    for nci, (n0, nl) in enumerate(LCH):
                        nc.scalar.activation(
                            ctxsb[:65, n0 : n0 + nl], cps3[nci][:, :nl], Act.Copy
                        )

                # ---- stage F: PE-transpose back, normalize, store ----
                for lc in range(9):
                    l0 = lc * 128
                    ll = min(128, L - l0)
                    osb = outp.tile([128, 384], F32, tag="osb", name=f"osb{b}_{lc}")
                    for h in range(6):
                        tp = ptp.tile([128, 65], F16, tag="tp", name=f"tpc{b}_{lc}_{h}")
                        nc.tensor.transpose(
                            tp[:ll, :],
                            ctx_tiles[h][:65, l0 : l0 + ll],
                            ident[:65, :65],
                        )
                        rec = small.tile([128, 1], F32, tag="rec", name=f"rec{b}_{lc}_{h}")
                        nc.vector.reciprocal(rec[:ll], tp[:ll, 64:65])
                        nc.vector.tensor_scalar_mul(
                            osb[:ll, h * 64 : (h + 1) * 64], tp[:ll, 0:64], rec[:ll]
                        )
                    nc.sync.dma_start(
                        out=out[b, l0 : l0 + ll, :], in_=osb[:ll, :]
                    )

            # software pipeline: batch b's conv/proj is emitted before batch
            # b-1's attention-dependent work drains, giving the PE dense
            # filler while ACT computes the exps.
            state = {}
            for b in range(BPC):
                state[b] = emit_load(b)
                if b >= 1:
                    emit_attn(b - 1, state.pop(b - 1))
                emit_convproj(b, state[b])
            emit_attn(BPC - 1, state.pop(BPC - 1))
    return nc


def _install_trace_support():
    """Provide the NTFF profile hook (this image's antenv lacks axon_hooks)
    and neuter the artifact upload (no fish access here)."""
    import contextlib
    import ctypes
    import types

    import concourse.bass_utils as bu

    bu.upload_artifacts = lambda tmpdir: f"local:{tmpdir}"
    try:
        from antenv.axon_hooks import get_axon_ntff_profile_hook  # noqa: F401

        return
    except ImportError:
        pass
    so_path = "/opt/axon/libaxon_pjrt.so"
    lib = ctypes.CDLL(so_path)
    if not hasattr(lib, "axon_start_nrt_profile"):
        return
    lib.axon_start_nrt_profile.argtypes = [
        ctypes.POINTER(ctypes.c_int64),
        ctypes.c_size_t,
    ]
    lib.axon_start_nrt_profile.restype = ctypes.c_int64
    lib.axon_stop_nrt_profile.argtypes = [ctypes.c_char_p]
    lib.axon_stop_nrt_profile.restype = ctypes.c_int64

    @contextlib.contextmanager
    def _hook(output_dir, device_ids):
        import jax

        jax.devices()
        if device_ids:
            ids = (ctypes.c_int64 * len(device_ids))(*device_ids)
            rc = lib.axon_start_nrt_profile(ids, len(device_ids))
        else:
            rc = lib.axon_start_nrt_profile(None, 0)
        if rc != 0:
            raise RuntimeError(f"axon_start_nrt_profile rc={rc}")
        try:
            yield
        finally:
            n = lib.axon_stop_nrt_profile(str(output_dir).encode())
            print(f"profile: {n} file(s) written to {output_dir}")

    import antenv

    mod = types.ModuleType("antenv.axon_hooks")
    holder = {"h": _hook}
    mod.get_axon_ntff_profile_hook = lambda: holder["h"]
    mod.set_axon_ntff_profile_hook = lambda h: holder.__setitem__("h", h)
    antenv.axon_hooks = mod
    sys.modules["antenv.axon_hooks"] = mod


_CACHED = None


def _prep_weights(inputs):
    """Fold BN into conv weights; pre-transpose/chunk projection weights."""
    f16 = np.float16
    wdiag = np.zeros((128, 81, 128), dtype=f16)
    biases = np.zeros((128, 15), dtype=np.float32)
    wproj = np.zeros((128, 18, 128), dtype=f16)
    wpv = np.zeros((128, 3, 384), dtype=f16)
    for ci, p in enumerate(["q", "k", "v"]):
        gamma = np.asarray(inputs[f"bn_{p}_gamma"], np.float64)
        var = np.asarray(inputs[f"bn_{p}_var"], np.float64)
        beta = np.asarray(inputs[f"bn_{p}_beta"], np.float64)
        mean = np.asarray(inputs[f"bn_{p}_mean"], np.float64)
        inv = gamma / np.sqrt(var + EPS)
        wfold = np.asarray(inputs[f"conv_{p}_w"], np.float64)[:, 0] * inv[:, None, None]
        bias_c = beta - mean * inv
        for tap in range(9):
            di, dj = tap // 3, tap % 3
            for cc in range(3):
                d = wfold[cc * 128 : (cc + 1) * 128, di, dj]
                np.fill_diagonal(wdiag[:, ci * 27 + tap * 3 + cc, :], d.astype(f16))
        for cc in range(3):
            biases[:, ci * 3 + cc] = bias_c[cc * 128 : (cc + 1) * 128]
        w = np.asarray(inputs[f"w_{p}"], np.float64)  # [o, c]
        assert np.abs(np.asarray(inputs[f"b_{p}"])).max() == 0.0 or p != "v", (
            "nonzero v bias unsupported"
        )
        if p == "q":
            wt = (w.T * (C**-0.5)).astype(f16)  # fold attention scale
        else:
            wt = w.T.astype(f16)
        if p in ("q", "k"):
            pi = 0 if p == "q" else 1
            for kc in range(3):
                for mc in range(3):
                    wproj[:, pi * 9 + kc * 3 + mc, :] = wt[
                        kc * 128 : (kc + 1) * 128, mc * 128 : (mc + 1) * 128
                    ]
            # projection bias (spec: zeros, but supported per out-channel)
            bvec = np.asarray(inputs[f"b_{p}"], np.float64) * (
                (C**-0.5) if p == "q" else 1.0
            )
            for mc in range(3):
                biases[:, 9 + pi * 3 + mc] = bvec[mc * 128 : (mc + 1) * 128]
        else:
            for kc in range(3):
                wpv[:, kc, :] = wt[kc * 128 : (kc + 1) * 128, :]
    return wdiag, wproj, wpv, biases


def kernel(**inputs) -> np.ndarray:
    global _CACHED, LAST_EXEC_NS
    from concourse.bass_utils import run_bass_kernel_spmd

    if TRACE:
        _install_trace_support()
    hidden = np.ascontiguousarray(np.asarray(inputs["hidden_state"], np.float32))
    assert hidden.shape == (B, L, C)
    wdiag, wproj, wpv, biases = _prep_weights(inputs)

    if _CACHED is None:
        _CACHED = _build_kernel()
    nc = _CACHED

    in_maps = []
    for core in range(NCORES):
        in_maps.append(
            {
                "hid": hidden[core * BPC : (core + 1) * BPC],
                "wdiag": wdiag,
                "wproj": wproj,
                "wpv": wpv,
                "biases": biases,
            }
        )
    res = run_bass_kernel_spmd(
        nc, in_maps, core_ids=list(range(NCORES)), trace=TRACE
    )
    LAST_EXEC_NS = res.exec_time_ns
    out = np.concatenate([res.results[i]["out"] for i in range(NCORES)], axis=0)
    return out.astype(np.float32)


# revision 23
# speedup vs baseline: 3.6536x; 1.0050x over previous
"""CvT self-attention (depthwise-conv QKV projection + MHA) on 8 Trainium2 cores.

Sharding: data-parallel over batch B=64 -> 8 batches per core. No collectives.

Per-core pipeline (per batch, all matmuls fp16 w/ fp32 PSUM accumulation):
  1. DMA hidden [1025, 384] fp32, convert fp16, PE-transpose to channel-major
     x_pad [c, 34, 34] (zero-padded spatially).
  2. Depthwise 3x3 conv + folded BN as 9 diagonal-matmul taps accumulating in
     PSUM (q: stride 1, k/v: stride 2 via strided access patterns).
  3. QKV linear projections. q/k produce [c_out, tokens]; v is computed with
     conv output as the stationary operand producing token-major [t, c_out].
  4. Attention per head, scores TRANSPOSED ([t, l]) so no transpose is needed
     between softmax and PV: scoresT = kh^T qh, exp (no max subtraction --
     scores are O(1)), PV with ones-augmented V so the softmax denominator
     falls out of the same matmul, then PE-transpose [65, l] -> [l, 65] and
     normalize by the denominator column.
"""

import sys

sys.path.insert(0, "/opt/trn_rl_repo")

import numpy as np

import concourse.bass as bass
import concourse.mybir as mybir
import concourse.tile as tile
from concourse.masks import make_identity
from concourse.vector_clock import ScopedClock

B, C, H, W = 64, 384, 32, 32
NH, HD = 6, 64
L = 1 + H * W  # 1025 query tokens
TK = 1 + (H // 2) * (W // 2)  # 257 key/value tokens
NCORES = 8
BPC = B // NCORES  # batches per core
EPS = 1e-5
F16 = mybir.dt.float16
F32 = mybir.dt.float32
Act = mybir.ActivationFunctionType

TRACE = False
LAST_EXEC_NS = None

# l chunks for the 1025-token free dim (balanced, PSUM bank = 512 fp32)
LCH = [(0, 342), (342, 342), (684, 341)]
# t chunks for the 257-token key dim over partitions
TCH = [(0, 128), (128, 128), (256, 1)]


def _split_multi_waits(nc):
    """walrus in this image only allows ONE sync wait per instruction. Move
    extra waits onto NoOps (same engine) inserted just before the offender."""
    from bass_rust import InstNoOp

    n_split = 0
    for blk in nc.m.functions[0].blocks:
        insts = blk.instructions
        out_list = []
        changed = False
        for inst in insts:
            si = inst.sync_info
            waits = list(si.on_wait) if si and si.on_wait else []
            if len(waits) > 1:
                changed = True
                for w in waits[:-1]:
                    n_split += 1
                    nop = InstNoOp(name=f"I-waitsplit-{n_split}", ins=[], outs=[])
                    nop.engine = inst.engine
                    nop.sync_info = mybir.SyncInfo(on_wait=[w], on_update=[])
                    out_list.append(nop)
                si.on_wait = waits[-1:]
            out_list.append(inst)
        if changed:
            blk.instructions = out_list


def _refuse_ldweights(nc):
    """Tile's legalizer pre-splits every matmul into LDWEIGHTS + MATMUL, but
    the InstMatmult still carries the weights AP. Drop all explicit LDWs
    (moving their waits to the next PE instruction) and let walrus --
    with --enable-ldw-opt=true -- manage weight loads itself (dedup +
    background-buffer overlap)."""
    removed = 0
    for blk in nc.m.functions[0].blocks:
        insts = blk.instructions
        out_list = []
        pending = []
        changed = False
        for inst in insts:
            if type(inst).__name__ == "InstLdweights":
                si = inst.sync_info
                if si and si.on_wait:
                    pending.extend(list(si.on_wait))
                removed += 1
                changed = True
                continue
            if pending and inst.engine == mybir.EngineType.PE:
                si = inst.sync_info
                if si is None:
                    inst.sync_info = mybir.SyncInfo(on_wait=pending, on_update=[])
                else:
                    si.on_wait = list(si.on_wait or []) + pending
                pending = []
            out_list.append(inst)
        if changed:
            blk.instructions = out_list
    return removed


def _patch_ldw_opt():
    """Let walrus dedup/overlap LDWEIGHTS (requires self-loading matmuls)."""
    import concourse.bass_utils as bu

    if getattr(bu, "_ldw_patched", False):
        return
    orig = bu.run_command

    def run_command_ldw(argv, **kw):
        argv = [
            "--enable-ldw-opt=true" if a == "--enable-ldw-opt=false" else a
            for a in argv
        ]
        return orig(argv, **kw)

    bu.run_command = run_command_ldw
    bu._ldw_patched = True


def _patch_drain():
    """Append wait-splitting to the end of TileContext's tail drain."""
    if getattr(tile.TileContext, "_drain_patched", False):
        return

    def _drain_and_barrier(self, tick_clock, wait_clock):
        nc = self.nc
        drain_inst = nc.sync.drain()
        wait_clock.add_sem_waits(
            drain_inst.ins, ScopedClock({None: tick_clock.global_clock})
        )
        nc.all_engine_barrier()
        assert self.sems is not None
        popped = nc._tile_sem_poison_stack.pop()
        assert popped is self._sem_poison
        nc.clear_and_free_semaphores(list(self.sems.allocated().values()))
        nc.all_engine_barrier()
        _split_multi_waits(nc)

    tile.TileContext._drain_and_barrier = _drain_and_barrier
    tile.TileContext._drain_patched = True


def _build_kernel():
    _patch_drain()
    nc = bass.Bass()
    hid = nc.dram_tensor("hid", [BPC, L, C], F32, kind="ExternalInput").ap()
    wdiag = nc.dram_tensor("wdiag", [128, 81, 128], F16, kind="ExternalInput").ap()
    wproj = nc.dram_tensor("wproj", [128, 18, 128], F16, kind="ExternalInput").ap()
    wpv = nc.dram_tensor("wpv", [128, 3, 384], F16, kind="ExternalInput").ap()
    biases = nc.dram_tensor("biases", [128, 15], F32, kind="ExternalInput").ap()
    out = nc.dram_tensor("out", [BPC, L, C], F32, kind="ExternalOutput").ap()

    with tile.TileContext(nc) as tc:
        with (
            tc.tile_pool(name="const", bufs=1) as const,
            tc.tile_pool(name="io", bufs=3) as io,
            tc.tile_pool(name="stage", bufs=2) as stage,
            tc.tile_pool(name="ctx", bufs=8) as ctxp,
            tc.tile_pool(name="outp", bufs=3) as outp,
            tc.tile_pool(name="small", bufs=4) as small,
            tc.tile_pool(name="pmm", bufs=4, space="PSUM") as pmm,
            tc.tile_pool(name="pctx", bufs=3, space="PSUM") as pctx,
            tc.tile_pool(name="ptp", bufs=1, space="PSUM") as ptp,
        ):
            # ---- constants ----
            wd_sb = const.tile([128, 81, 128], F16, tag="wd")
            nc.sync.dma_start(out=wd_sb[:], in_=wdiag)
            wp_sb = const.tile([128, 18, 128], F16, tag="wp")
            nc.sync.dma_start(out=wp_sb[:], in_=wproj)
            wpv_sb = const.tile([128, 3, 384], F16, tag="wpv")
            nc.sync.dma_start(out=wpv_sb[:], in_=wpv)
            bias_sb = const.tile([128, 15], F32, tag="bias")
            nc.sync.dma_start(out=bias_sb[:], in_=biases)
            ident = const.tile([128, 128], F16, tag="ident")
            make_identity(nc, ident[:])

            def emit_load(b):
                # ---- stage A: load (casting DMA) + PE-transpose to channel-major ----
                x_pad = stage.tile([128, 3, 34, 34], F16, tag="xpad", name=f"xpad{b}")
                # zero the 1-px border (interior is fully overwritten)
                nc.gpsimd.memset(x_pad[:, :, 0, :], 0.0)
                nc.gpsimd.memset(x_pad[:, :, 33, :], 0.0)
                nc.gpsimd.memset(x_pad[:, :, 1:33, 0], 0.0)
                nc.gpsimd.memset(x_pad[:, :, 1:33, 33], 0.0)

                for k in range(8):
                    x16 = io.tile([128, 384], F16, tag="x16", name=f"x16_{b}_{k}")
                    nc.gpsimd.dma_start(
                        out=x16[:], in_=hid[b, 1 + 128 * k : 1 + 128 * (k + 1), :]
                    )
                    for cc in range(3):
                        tp = ptp.tile([128, 128], F16, tag="tp", name=f"tpi{b}_{k}_{cc}")
                        nc.tensor.transpose(
                            tp[:], x16[:, cc * 128 : (cc + 1) * 128], ident[:]
                        )
                        # tokens 128k..128k+127 = image rows 4k..4k+3
                        nc.vector.tensor_copy(
                            x_pad[:, cc, 1 + 4 * k : 5 + 4 * k, 1:33], tp[:]
                        )

                cls16 = small.tile([128, 3], F16, tag="cls", name=f"cls{b}")
                for cc in range(3):
                    nc.gpsimd.dma_start(
                        out=cls16[:, cc : cc + 1],
                        in_=hid[b, 0:1, cc * 128 : (cc + 1) * 128].rearrange(
                            "a b -> b a"
                        ),
                    )
                return {"x_pad": x_pad, "cls16": cls16}

            def emit_convproj(b, st):
                x_pad, cls16 = st["x_pad"], st["cls16"]
                # ---- stage B: depthwise conv + BN (diagonal matmuls) ----
                q_src = stage.tile([128, 3, 1025], F16, tag="qsrc", name=f"qsrc{b}")
                k_src = stage.tile([128, 3, 257], F16, tag="ksrc", name=f"ksrc{b}")
                v_src = stage.tile([128, 3, 257], F16, tag="vsrc", name=f"vsrc{b}")
                for cc in range(3):
                    for s in (q_src, k_src, v_src):
                        nc.gpsimd.tensor_copy(s[:, cc, 0:1], cls16[:, cc : cc + 1])
                for cc in range(3):
                    # q: stride 1, two 512-token banks; taps outer so both
                    # banks' matmuls share one LDWEIGHTS
                    psq = [
                        pmm.tile([128, 512], F32, tag="mm", name=f"psq{b}_{cc}_{i}")
                        for i in range(2)
                    ]
                    for tap in range(9):
                        di, dj = tap // 3, tap % 3
                        for nb in range(2):
                            rhs = x_pad[
                                :, cc, 16 * nb + di : 16 * nb + di + 16, dj : dj + 32
                            ]
                            nc.tensor.matmul(
                                psq[nb][:],
                                wd_sb[:, tap * 3 + cc, :],
                                rhs,
                                start=(tap == 0),
                                stop=(tap == 8),
                            )
                    for nb in range(2):
                        nc.scalar.activation(
                            q_src[:, cc, 1 + 512 * nb : 513 + 512 * nb],
                            psq[nb][:],
                            Act.Identity,
                            bias=bias_sb[:, cc : cc + 1],
                        )
                    # k, v: stride 2 (16x16 outputs)
                    xv = x_pad[:, cc].rearrange(
                        "p (i ti) (j tj) -> p i ti j tj", ti=2, tj=2
                    )
                    for ci, src in ((1, k_src), (2, v_src)):
                        ps = pmm.tile([128, 512], F32, tag="mm", name=f"pkv{b}_{cc}_{ci}")
                        for tap in range(9):
                            di, dj = tap // 3, tap % 3
                            rhs = xv[
                                :,
                                di // 2 : di // 2 + 16,
                                di % 2,
                                dj // 2 : dj // 2 + 16,
                                dj % 2,
                            ]
                            nc.tensor.matmul(
                                ps[:, :256],
                                wd_sb[:, ci * 27 + tap * 3 + cc, :],
                                rhs,
                                start=(tap == 0),
                                stop=(tap == 8),
                            )
                        nc.scalar.activation(
                            src[:, cc, 1:257],
                            ps[:, :256],
                            Act.Identity,
                            bias=bias_sb[:, ci * 3 + cc : ci * 3 + cc + 1],
                        )

                # ---- stage C: projections (kc inner-adjacent for LDW reuse) ----
                qh = stage.tile([128, 3, 1025], F16, tag="qh", name=f"qh{b}")
                kh = stage.tile([128, 3, 257], F16, tag="kh", name=f"kh{b}")
                for mc in range(3):
                    ps3 = [
                        pmm.tile([128, 512], F32, tag="mm", name=f"ps3_{b}_{mc}_{i}")
                        for i in range(3)
                    ]
                    for kc in range(3):
                        for nci, (n0, nl) in enumerate(LCH):
                            nc.tensor.matmul(
                                ps3[nci][:, :nl],
                                wp_sb[:, kc * 3 + mc, :],
                                q_src[:, kc, n0 : n0 + nl],
                                start=(kc == 0),
                                stop=(kc == 2),
                            )
                    for nci, (n0, nl) in enumerate(LCH):
                        nc.vector.tensor_scalar_add(
                            qh[:, mc, n0 : n0 + nl],
                            ps3[nci][:, :nl],
                            bias_sb[:, 9 + mc : 10 + mc],
                        )
                    ps = pmm.tile([128, 512], F32, tag="mm", name=f"pk{b}_{mc}")
                    for kc in range(3):
                        nc.tensor.matmul(
                            ps[:, :257],
                            wp_sb[:, 9 + kc * 3 + mc, :],
                            k_src[:, kc, :],
                            start=(kc == 0),
                            stop=(kc == 2),
                        )
                    nc.vector.tensor_scalar_add(
                        kh[:, mc, :], ps[:, :257], bias_sb[:, 12 + mc : 13 + mc]
                    )
                v_store = stage.tile([128, 3, 6, 65], F16, tag="vst", name=f"vst{b}")
                nc.gpsimd.memset(v_store[:, :, :, 64:65], 1.0)
                for tcc, (t0, tl) in enumerate(TCH):
                    ps = pmm.tile([128, 512], F32, tag="mm", name=f"pv{b}_{tcc}")
                    for kc in range(3):
                        nc.tensor.matmul(
                            ps[:tl, :384],
                            v_src[:, kc, t0 : t0 + tl],
                            wpv_sb[:, kc, :],
                            start=(kc == 0),
                            stop=(kc == 2),
                        )
                    nc.vector.tensor_copy(
                        v_store[:tl, tcc, :, 0:64],
                        ps[:tl, :384].rearrange("p (h d) -> p h d", h=6),
                    )
                st.update(qh=qh, kh=kh, v_store=v_store)

            def emit_attn(b, st):
                qh, kh, v_store = st["qh"], st["kh"], st["v_store"]
                # ---- stage D/E: attention, heads in pairs. The pair lives
                # at PE row-groups 0-63 / 64-127, so its score matmuls run
                # CONCURRENTLY on the array (row tiling) when interleaved.
                ctx_tiles = [None] * 6
                for hp in range(3):
                    h0, h1 = 2 * hp, 2 * hp + 1
                    ch = hp
                    exps = [
                        stage.tile(
                            [128, 3, 1025], F16, tag="expT", bufs=4,
                            name=f"expT{b}_{h}",
                        )
                        for h in (h0, h1)
                    ]
                    for tcc, (t0, tl) in enumerate(TCH):
                        for n0, nl in LCH:
                            pspair = [
                                pmm.tile(
                                    [128, 512], F32, tag="mm",
                                    name=f"sc{b}_{h}_{tcc}_{n0}",
                                )
                                for h in (h0, h1)
                            ]
                            for side in range(2):
                                base = side * 64
                                nc.tensor.matmul(
                                    pspair[side][:tl, :nl],
                                    kh[base : base + 64, ch, t0 : t0 + tl],
                                    qh[base : base + 64, ch, n0 : n0 + nl],
                                    start=True,
                                    stop=True,
                                )
                            for side in range(2):
                                nc.scalar.activation(
                                    exps[side][:tl, tcc, n0 : n0 + nl],
                                    pspair[side][:tl, :nl],
                                    Act.Exp,
                                )
                    # PV: tc outer so the 3 l-chunks share each LDWEIGHTS
                    for side, h in ((0, h0), (1, h1)):
                        ctxsb = ctxp.tile(
                            [128, 1152], F16, tag="ctx", name=f"ctx{b}_{h}"
                        )
                        ctx_tiles[h] = ctxsb
                        cps3 = [
                            pctx.tile([65, 512], F32, tag="pc", name=f"cps{b}_{h}_{i}")
                            for i in range(3)
                        ]
                        for tcc, (t0, tl) in enumerate(TCH):
                            for nci, (n0, nl) in enumerate(LCH):
                                nc.tensor.matmul(
                                    cps3[nci][:, :nl],
                                    v_store[:tl, tcc, h, :],
                                    exps[side][:tl, tcc, n0 : n0 + nl],
                                    start=(tcc == 0),
                                    stop=(tcc == 2),
                                )
                        for nci, (n0, nl) in enumerate(LCH):
                            nc.scalar.activation(
                                ctxsb[:65, n0 : n0 + nl], cps3[nci][:, :nl], Act.Copy
                            )

                # ---- stage F: PE-transpose back, normalize, store ----
                for lc in range(9):
                    l0 = lc * 128
                    ll = min(128, L - l0)
                    osb = outp.tile([128, 384], F32, tag="osb", name=f"osb{b}_{lc}")
                    for h in range(6):
                        tp = ptp.tile([128, 65], F16, tag="tp", name=f"tpc{b}_{lc}_{h}")
                        nc.tensor.transpose(
                            tp[:ll, :],
                            ctx_tiles[h][:65, l0 : l0 + ll],
                            ident[:65, :65],
                        )
                        rec = small.tile([128, 1], F32, tag="rec", name=f"rec{b}_{lc}_{h}")
                        nc.vector.reciprocal(rec[:ll], tp[:ll, 64:65])
                        nc.vector.tensor_scalar_mul(
                            osb[:ll, h * 64 : (h + 1) * 64], tp[:ll, 0:64], rec[:ll]
                        )
                    nc.sync.dma_start(
                        out=out[b, l0 : l0 + ll, :], in_=osb[:ll, :]
                    )

            # software pipeline: batch b's conv/proj is emitted before batch
            # b-1's attention-dependent work drains, giving the PE dense
            # filler while ACT computes the exps.
            state = {}
            for b in range(BPC):
                state[b] = emit_load(b)
                if b >= 1:
                    emit_attn(b - 1, state.pop(b - 1))
                emit_convproj(b, state[b])
            emit_attn(BPC - 1, state.pop(BPC - 1))
    return nc


def _install_trace_support():
    """Provide the NTFF profile hook (this image's antenv lacks axon_hooks)
    and neuter the artifact upload (no fish access here)."""
    import contextlib
    import ctypes
    import types

    import concourse.bass_utils as bu

    bu.upload_artifacts = lambda tmpdir: f"local:{tmpdir}"
    try:
        from antenv.axon_hooks import get_axon_ntff_profile_hook  # noqa: F401

        return
    except ImportError:
        pass
    so_path = "/opt/axon/libaxon_pjrt.so"
    lib = ctypes.CDLL(so_path)
    if not hasattr(lib, "axon_start_nrt_profile"):
        return
    lib.axon_start_nrt_profile.argtypes = [
        ctypes.POINTER(ctypes.c_int64),
        ctypes.c_size_t,
    ]
    lib.axon_start_nrt_profile.restype = ctypes.c_int64
    lib.axon_stop_nrt_profile.argtypes = [ctypes.c_char_p]
    lib.axon_stop_nrt_profile.restype = ctypes.c_int64

    @contextlib.contextmanager
    def _hook(output_dir, device_ids):
        import jax

        jax.devices()
        if device_ids:
            ids = (ctypes.c_int64 * len(device_ids))(*device_ids)
            rc = lib.axon_start_nrt_profile(ids, len(device_ids))
        else:
            rc = lib.axon_start_nrt_profile(None, 0)
        if rc != 0:
            raise RuntimeError(f"axon_start_nrt_profile rc={rc}")
        try:
            yield
        finally:
            n = lib.axon_stop_nrt_profile(str(output_dir).encode())
            print(f"profile: {n} file(s) written to {output_dir}")

    import antenv

    mod = types.ModuleType("antenv.axon_hooks")
    holder = {"h": _hook}
    mod.get_axon_ntff_profile_hook = lambda: holder["h"]
    mod.set_axon_ntff_profile_hook = lambda h: holder.__setitem__("h", h)
    antenv.axon_hooks = mod
    sys.modules["antenv.axon_hooks"] = mod


_CACHED = None


def _prep_weights(inputs):
    """Fold BN into conv weights; pre-transpose/chunk projection weights."""
    f16 = np.float16
    wdiag = np.zeros((128, 81, 128), dtype=f16)
    biases = np.zeros((128, 15), dtype=np.float32)
    wproj = np.zeros((128, 18, 128), dtype=f16)
    wpv = np.zeros((128, 3, 384), dtype=f16)
    for ci, p in enumerate(["q", "k", "v"]):
        gamma = np.asarray(inputs[f"bn_{p}_gamma"], np.float64)
        var = np.asarray(inputs[f"bn_{p}_var"], np.float64)
        beta = np.asarray(inputs[f"bn_{p}_beta"], np.float64)
        mean = np.asarray(inputs[f"bn_{p}_mean"], np.float64)
        inv = gamma / np.sqrt(var + EPS)
        wfold = np.asarray(inputs[f"conv_{p}_w"], np.float64)[:, 0] * inv[:, None, None]
        bias_c = beta - mean * inv
        for tap in range(9):
            di, dj = tap // 3, tap % 3
            for cc in range(3):
                d = wfold[cc * 128 : (cc + 1) * 128, di, dj]
                np.fill_diagonal(wdiag[:, ci * 27 + tap * 3 + cc, :], d.astype(f16))
        for cc in range(3):
            biases[:, ci * 3 + cc] = bias_c[cc * 128 : (cc + 1) * 128]
        w = np.asarray(inputs[f"w_{p}"], np.float64)  # [o, c]
        assert np.abs(np.asarray(inputs[f"b_{p}"])).max() == 0.0 or p != "v", (
            "nonzero v bias unsupported"
        )
        if p == "q":
            wt = (w.T * (C**-0.5)).astype(f16)  # fold attention scale
        else:
            wt = w.T.astype(f16)
        if p in ("q", "k"):
            pi = 0 if p == "q" else 1
            for kc in range(3):
                for mc in range(3):
                    wproj[:, pi * 9 + kc * 3 + mc, :] = wt[
                        kc * 128 : (kc + 1) * 128, mc * 128 : (mc + 1) * 128
                    ]
            # projection bias (spec: zeros, but supported per out-channel)
            bvec = np.asarray(inputs[f"b_{p}"], np.float64) * (
                (C**-0.5) if p == "q" else 1.0
            )
            for mc in range(3):
                biases[:, 9 + pi * 3 + mc] = bvec[mc * 128 : (mc + 1) * 128]
        else:
            for kc in range(3):
                wpv[:, kc, :] = wt[kc * 128 : (kc + 1) * 128, :]
    return wdiag, wproj, wpv, biases


def kernel(**inputs) -> np.ndarray:
    global _CACHED, LAST_EXEC_NS
    from concourse.bass_utils import run_bass_kernel_spmd

    if TRACE:
        _install_trace_support()
    hidden = np.ascontiguousarray(np.asarray(inputs["hidden_state"], np.float32))
    assert hidden.shape == (B, L, C)
    wdiag, wproj, wpv, biases = _prep_weights(inputs)

    if _CACHED is None:
        _CACHED = _build_kernel()
    nc = _CACHED

    in_maps = []
    for core in range(NCORES):
        in_maps.append(
            {
                "hid": hidden[core * BPC : (core + 1) * BPC],
                "wdiag": wdiag,
                "wproj": wproj,
                "wpv": wpv,
                "biases": biases,
            }
        )
    res = run_bass_kernel_spmd(
        nc, in_maps, core_ids=list(range(NCORES)), trace=TRACE
    )
    LAST_EXEC_NS = res.exec_time_ns
    out = np.concatenate([res.results[i]["out"] for i in range(NCORES)], axis=0)
    return out.astype(np.float32)


# revision 24
# speedup vs baseline: 4.0418x; 1.1062x over previous
"""CvT self-attention (depthwise-conv QKV projection + MHA) on 8 Trainium2 cores.

Sharding: data-parallel over batch B=64 -> 8 batches per core. No collectives.

Per-core pipeline (per batch, all matmuls fp16 w/ fp32 PSUM accumulation):
  1. DMA hidden [1025, 384] fp32, convert fp16, PE-transpose to channel-major
     x_pad [c, 34, 34] (zero-padded spatially).
  2. Depthwise 3x3 conv + folded BN as 9 diagonal-matmul taps accumulating in
     PSUM (q: stride 1, k/v: stride 2 via strided access patterns).
  3. QKV linear projections. q/k produce [c_out, tokens]; v is computed with
     conv output as the stationary operand producing token-major [t, c_out].
  4. Attention per head, scores TRANSPOSED ([t, l]) so no transpose is needed
     between softmax and PV: scoresT = kh^T qh, exp (no max subtraction --
     scores are O(1)), PV with ones-augmented V so the softmax denominator
     falls out of the same matmul, then PE-transpose [65, l] -> [l, 65] and
     normalize by the denominator column.
"""

import sys

sys.path.insert(0, "/opt/trn_rl_repo")

import numpy as np

import concourse.bass as bass
import concourse.mybir as mybir
import concourse.tile as tile
from concourse.masks import make_identity
from concourse.vector_clock import ScopedClock

B, C, H, W = 64, 384, 32, 32
NH, HD = 6, 64
L = 1 + H * W  # 1025 query tokens
TK = 1 + (H // 2) * (W // 2)  # 257 key/value tokens
NCORES = 8
BPC = B // NCORES  # batches per core
EPS = 1e-5
F16 = mybir.dt.float16
F32 = mybir.dt.float32
Act = mybir.ActivationFunctionType

TRACE = False
LAST_EXEC_NS = None

# l chunks for the 1025-token free dim (balanced, PSUM bank = 512 fp32)
LCH = [(0, 342), (342, 342), (684, 341)]
# t chunks for the 257-token key dim over partitions
TCH = [(0, 128), (128, 128), (256, 1)]


def _split_multi_waits(nc):
    """walrus in this image only allows ONE sync wait per instruction. Move
    extra waits onto NoOps (same engine) inserted just before the offender."""
    from bass_rust import InstNoOp

    n_split = 0
    for blk in nc.m.functions[0].blocks:
        insts = blk.instructions
        out_list = []
        changed = False
        for inst in insts:
            si = inst.sync_info
            waits = list(si.on_wait) if si and si.on_wait else []
            if len(waits) > 1:
                changed = True
                for w in waits[:-1]:
                    n_split += 1
                    nop = InstNoOp(name=f"I-waitsplit-{n_split}", ins=[], outs=[])
                    nop.engine = inst.engine
                    nop.sync_info = mybir.SyncInfo(on_wait=[w], on_update=[])
                    out_list.append(nop)
                si.on_wait = waits[-1:]
            out_list.append(inst)
        if changed:
            blk.instructions = out_list


def _refuse_ldweights(nc):
    """Tile's legalizer pre-splits every matmul into LDWEIGHTS + MATMUL, but
    the InstMatmult still carries the weights AP. Drop all explicit LDWs
    (moving their waits to the next PE instruction) and let walrus --
    with --enable-ldw-opt=true -- manage weight loads itself (dedup +
    background-buffer overlap)."""
    removed = 0
    for blk in nc.m.functions[0].blocks:
        insts = blk.instructions
        out_list = []
        pending = []
        changed = False
        for inst in insts:
            if type(inst).__name__ == "InstLdweights":
                si = inst.sync_info
                if si and si.on_wait:
                    pending.extend(list(si.on_wait))
                removed += 1
                changed = True
                continue
            if pending and inst.engine == mybir.EngineType.PE:
                si = inst.sync_info
                if si is None:
                    inst.sync_info = mybir.SyncInfo(on_wait=pending, on_update=[])
                else:
                    si.on_wait = list(si.on_wait or []) + pending
                pending = []
            out_list.append(inst)
        if changed:
            blk.instructions = out_list
    return removed


def _patch_ldw_opt():
    """Let walrus dedup/overlap LDWEIGHTS (requires self-loading matmuls)."""
    import concourse.bass_utils as bu

    if getattr(bu, "_ldw_patched", False):
        return
    orig = bu.run_command

    def run_command_ldw(argv, **kw):
        argv = [
            "--enable-ldw-opt=true" if a == "--enable-ldw-opt=false" else a
            for a in argv
        ]
        return orig(argv, **kw)

    bu.run_command = run_command_ldw
    bu._ldw_patched = True


def _patch_drain():
    """Append wait-splitting to the end of TileContext's tail drain."""
    if getattr(tile.TileContext, "_drain_patched", False):
        return

    def _drain_and_barrier(self, tick_clock, wait_clock):
        nc = self.nc
        drain_inst = nc.sync.drain()
        wait_clock.add_sem_waits(
            drain_inst.ins, ScopedClock({None: tick_clock.global_clock})
        )
        nc.all_engine_barrier()
        assert self.sems is not None
        popped = nc._tile_sem_poison_stack.pop()
        assert popped is self._sem_poison
        nc.clear_and_free_semaphores(list(self.sems.allocated().values()))
        nc.all_engine_barrier()
        _split_multi_waits(nc)

    tile.TileContext._drain_and_barrier = _drain_and_barrier
    tile.TileContext._drain_patched = True


def _build_kernel():
    _patch_drain()
    nc = bass.Bass()
    hid = nc.dram_tensor("hid", [BPC, L, C], F32, kind="ExternalInput").ap()
    wdiag = nc.dram_tensor("wdiag", [128, 81, 128], F16, kind="ExternalInput").ap()
    wproj = nc.dram_tensor("wproj", [128, 18, 128], F16, kind="ExternalInput").ap()
    wpv = nc.dram_tensor("wpv", [128, 3, 384], F16, kind="ExternalInput").ap()
    biases = nc.dram_tensor("biases", [128, 15], F32, kind="ExternalInput").ap()
    out = nc.dram_tensor("out", [BPC, L, C], F32, kind="ExternalOutput").ap()

    with tile.TileContext(nc) as tc:
        with (
            tc.tile_pool(name="const", bufs=1) as const,
            tc.tile_pool(name="io", bufs=3) as io,
            tc.tile_pool(name="stage", bufs=2) as stage,
            tc.tile_pool(name="ctx", bufs=8) as ctxp,
            tc.tile_pool(name="outp", bufs=3) as outp,
            tc.tile_pool(name="small", bufs=4) as small,
            tc.tile_pool(name="pmm", bufs=4, space="PSUM") as pmm,
            tc.tile_pool(name="pctx", bufs=2, space="PSUM") as pctx,
            tc.tile_pool(name="ptp", bufs=2, space="PSUM") as ptp,
        ):
            # ---- constants ----
            wd_sb = const.tile([128, 81, 128], F16, tag="wd")
            nc.sync.dma_start(out=wd_sb[:], in_=wdiag)
            wp_sb = const.tile([128, 18, 128], F16, tag="wp")
            nc.sync.dma_start(out=wp_sb[:], in_=wproj)
            wpv_sb = const.tile([128, 3, 384], F16, tag="wpv")
            nc.sync.dma_start(out=wpv_sb[:], in_=wpv)
            bias_sb = const.tile([128, 15], F32, tag="bias")
            nc.sync.dma_start(out=bias_sb[:], in_=biases)
            ident = const.tile([128, 128], F16, tag="ident")
            make_identity(nc, ident[:])

            def emit_load(b):
                # ---- stage A: load (casting DMA) + PE-transpose to channel-major ----
                x_pad = stage.tile([128, 3, 34, 34], F16, tag="xpad", name=f"xpad{b}")
                # zero the 1-px border (interior is fully overwritten)
                nc.gpsimd.memset(x_pad[:, :, 0, :], 0.0)
                nc.gpsimd.memset(x_pad[:, :, 33, :], 0.0)
                nc.gpsimd.memset(x_pad[:, :, 1:33, 0], 0.0)
                nc.gpsimd.memset(x_pad[:, :, 1:33, 33], 0.0)

                for k in range(8):
                    x16 = io.tile([128, 384], F16, tag="x16", name=f"x16_{b}_{k}")
                    nc.gpsimd.dma_start(
                        out=x16[:], in_=hid[b, 1 + 128 * k : 1 + 128 * (k + 1), :]
                    )
                    for cc in range(3):
                        tp = ptp.tile([128, 128], F16, tag="tp", name=f"tpi{b}_{k}_{cc}")
                        nc.tensor.transpose(
                            tp[:], x16[:, cc * 128 : (cc + 1) * 128], ident[:]
                        )
                        # tokens 128k..128k+127 = image rows 4k..4k+3
                        nc.vector.tensor_copy(
                            x_pad[:, cc, 1 + 4 * k : 5 + 4 * k, 1:33], tp[:]
                        )

                cls16 = small.tile([128, 3], F16, tag="cls", name=f"cls{b}")
                for cc in range(3):
                    nc.gpsimd.dma_start(
                        out=cls16[:, cc : cc + 1],
                        in_=hid[b, 0:1, cc * 128 : (cc + 1) * 128].rearrange(
                            "a b -> b a"
                        ),
                    )
                return {"x_pad": x_pad, "cls16": cls16}

            def emit_convproj(b, st):
                x_pad, cls16 = st["x_pad"], st["cls16"]
                # ---- stage B: depthwise conv + BN (diagonal matmuls) ----
                q_src = stage.tile([128, 3, 1025], F16, tag="qsrc", name=f"qsrc{b}")
                k_src = stage.tile([128, 3, 257], F16, tag="ksrc", name=f"ksrc{b}")
                v_src = stage.tile([128, 3, 257], F16, tag="vsrc", name=f"vsrc{b}")
                for cc in range(3):
                    for s in (q_src, k_src, v_src):
                        nc.gpsimd.tensor_copy(s[:, cc, 0:1], cls16[:, cc : cc + 1])
                for cc in range(3):
                    # q: stride 1, two 512-token banks; taps outer so both
                    # banks' matmuls share one LDWEIGHTS
                    psq = [
                        pmm.tile([128, 512], F32, tag="mm", name=f"psq{b}_{cc}_{i}")
                        for i in range(2)
                    ]
                    for tap in range(9):
                        di, dj = tap // 3, tap % 3
                        for nb in range(2):
                            rhs = x_pad[
                                :, cc, 16 * nb + di : 16 * nb + di + 16, dj : dj + 32
                            ]
                            nc.tensor.matmul(
                                psq[nb][:],
                                wd_sb[:, tap * 3 + cc, :],
                                rhs,
                                start=(tap == 0),
                                stop=(tap == 8),
                            )
                    for nb in range(2):
                        nc.scalar.activation(
                            q_src[:, cc, 1 + 512 * nb : 513 + 512 * nb],
                            psq[nb][:],
                            Act.Identity,
                            bias=bias_sb[:, cc : cc + 1],
                        )
                    # k, v: stride 2 (16x16 outputs)
                    xv = x_pad[:, cc].rearrange(
                        "p (i ti) (j tj) -> p i ti j tj", ti=2, tj=2
                    )
                    for ci, src in ((1, k_src), (2, v_src)):
                        ps = pmm.tile([128, 512], F32, tag="mm", name=f"pkv{b}_{cc}_{ci}")
                        for tap in range(9):
                            di, dj = tap // 3, tap % 3
                            rhs = xv[
                                :,
                                di // 2 : di // 2 + 16,
                                di % 2,
                                dj // 2 : dj // 2 + 16,
                                dj % 2,
                            ]
                            nc.tensor.matmul(
                                ps[:, :256],
                                wd_sb[:, ci * 27 + tap * 3 + cc, :],
                                rhs,
                                start=(tap == 0),
                                stop=(tap == 8),
                            )
                        nc.scalar.activation(
                            src[:, cc, 1:257],
                            ps[:, :256],
                            Act.Identity,
                            bias=bias_sb[:, ci * 3 + cc : ci * 3 + cc + 1],
                        )

                # ---- stage C: projections (kc inner-adjacent for LDW reuse) ----
                qh = stage.tile([128, 3, 1025], F16, tag="qh", name=f"qh{b}")
                kh = stage.tile([128, 3, 257], F16, tag="kh", name=f"kh{b}")
                for mc in range(3):
                    ps3 = [
                        pmm.tile([128, 512], F32, tag="mm", name=f"ps3_{b}_{mc}_{i}")
                        for i in range(3)
                    ]
                    for kc in range(3):
                        for nci, (n0, nl) in enumerate(LCH):
                            nc.tensor.matmul(
                                ps3[nci][:, :nl],
                                wp_sb[:, kc * 3 + mc, :],
                                q_src[:, kc, n0 : n0 + nl],
                                start=(kc == 0),
                                stop=(kc == 2),
                            )
                    for nci, (n0, nl) in enumerate(LCH):
                        nc.vector.tensor_scalar_add(
                            qh[:, mc, n0 : n0 + nl],
                            ps3[nci][:, :nl],
                            bias_sb[:, 9 + mc : 10 + mc],
                        )
                    ps = pmm.tile([128, 512], F32, tag="mm", name=f"pk{b}_{mc}")
                    for kc in range(3):
                        nc.tensor.matmul(
                            ps[:, :257],
                            wp_sb[:, 9 + kc * 3 + mc, :],
                            k_src[:, kc, :],
                            start=(kc == 0),
                            stop=(kc == 2),
                        )
                    nc.vector.tensor_scalar_add(
                        kh[:, mc, :], ps[:, :257], bias_sb[:, 12 + mc : 13 + mc]
                    )
                v_store = stage.tile([128, 3, 6, 65], F16, tag="vst", name=f"vst{b}")
                nc.gpsimd.memset(v_store[:, :, :, 64:65], 1.0)
                for tcc, (t0, tl) in enumerate(TCH):
                    ps = pmm.tile([128, 512], F32, tag="mm", name=f"pv{b}_{tcc}")
                    for kc in range(3):
                        nc.tensor.matmul(
                            ps[:tl, :384],
                            v_src[:, kc, t0 : t0 + tl],
                            wpv_sb[:, kc, :],
                            start=(kc == 0),
                            stop=(kc == 2),
                        )
                    nc.vector.tensor_copy(
                        v_store[:tl, tcc, :, 0:64],
                        ps[:tl, :384].rearrange("p (h d) -> p h d", h=6),
                    )
                st.update(qh=qh, kh=kh, v_store=v_store)

            def emit_attn(b, st):
                qh, kh, v_store = st["qh"], st["kh"], st["v_store"]
                # ---- stage D/E: attention, heads in pairs. The pair lives
                # at PE row-groups 0-63 / 64-127, so its score matmuls run
                # CONCURRENTLY on the array (row tiling) when interleaved.
                ctx_tiles = [None] * 6
                for hp in range(3):
                    h0, h1 = 2 * hp, 2 * hp + 1
                    ch = hp
                    exps = [
                        stage.tile(
                            [128, 3, 1025], F16, tag="expT", bufs=4,
                            name=f"expT{b}_{h}",
                        )
                        for h in (h0, h1)
                    ]
                    for tcc, (t0, tl) in enumerate(TCH):
                        for n0, nl in LCH:
                            pspair = [
                                pmm.tile(
                                    [128, 512], F32, tag="mm",
                                    name=f"sc{b}_{h}_{tcc}_{n0}",
                                )
                                for h in (h0, h1)
                            ]
                            for side in range(2):
                                base = side * 64
                                nc.tensor.matmul(
                                    pspair[side][:tl, :nl],
                                    kh[base : base + 64, ch, t0 : t0 + tl],
                                    qh[base : base + 64, ch, n0 : n0 + nl],
                                    start=True,
                                    stop=True,
                                )
                            for side in range(2):
                                nc.scalar.activation(
                                    exps[side][:tl, tcc, n0 : n0 + nl],
                                    pspair[side][:tl, :nl],
                                    Act.Exp,
                                )
                    # PV: tc outer so the 3 l-chunks share each LDWEIGHTS
                    for side, h in ((0, h0), (1, h1)):
                        ctxsb = ctxp.tile(
                            [128, 1152], F16, tag="ctx", name=f"ctx{b}_{h}"
                        )
                        ctx_tiles[h] = ctxsb
                        for nci, (n0, nl) in enumerate(LCH):
                            cps = pctx.tile(
                                [65, 512], F32, tag="pc", name=f"cps{b}_{h}_{nci}"
                            )
                            for tcc, (t0, tl) in enumerate(TCH):
                                nc.tensor.matmul(
                                    cps[:, :nl],
                                    v_store[:tl, tcc, h, :],
                                    exps[side][:tl, tcc, n0 : n0 + nl],
                                    start=(tcc == 0),
                                    stop=(tcc == 2),
                                )
                            nc.scalar.activation(
                                ctxsb[:65, n0 : n0 + nl], cps[:, :nl], Act.Copy
                            )

                # ---- stage F: PE-transpose back, normalize, store ----
                for lc in range(9):
                    l0 = lc * 128
                    ll = min(128, L - l0)
                    osb = outp.tile([128, 384], F32, tag="osb", name=f"osb{b}_{lc}")
                    for h in range(6):
                        tp = ptp.tile([128, 65], F16, tag="tp", name=f"tpc{b}_{lc}_{h}")
                        nc.tensor.transpose(
                            tp[:ll, :],
                            ctx_tiles[h][:65, l0 : l0 + ll],
                            ident[:65, :65],
                        )
                        rec = small.tile([128, 1], F32, tag="rec", name=f"rec{b}_{lc}_{h}")
                        nc.vector.reciprocal(rec[:ll], tp[:ll, 64:65])
                        nc.vector.tensor_scalar_mul(
                            osb[:ll, h * 64 : (h + 1) * 64], tp[:ll, 0:64], rec[:ll]
                        )
                    nc.sync.dma_start(
                        out=out[b, l0 : l0 + ll, :], in_=osb[:ll, :]
                    )

            # software pipeline: batch b's conv/proj is emitted before batch
            # b-1's attention-dependent work drains, giving the PE dense
            # filler while ACT computes the exps.
            state = {}
            for b in range(BPC):
                state[b] = emit_load(b)
                if b >= 1:
                    emit_attn(b - 1, state.pop(b - 1))
                emit_convproj(b, state[b])
            emit_attn(BPC - 1, state.pop(BPC - 1))
    return nc


def _install_trace_support():
    """Provide the NTFF profile hook (this image's antenv lacks axon_hooks)
    and neuter the artifact upload (no fish access here)."""
    import contextlib
    import ctypes
    import types

    import concourse.bass_utils as bu

    bu.upload_artifacts = lambda tmpdir: f"local:{tmpdir}"
    try:
        from antenv.axon_hooks import get_axon_ntff_profile_hook  # noqa: F401

        return
    except ImportError:
        pass
    so_path = "/opt/axon/libaxon_pjrt.so"
    lib = ctypes.CDLL(so_path)
    if not hasattr(lib, "axon_start_nrt_profile"):
        return
    lib.axon_start_nrt_profile.argtypes = [
        ctypes.POINTER(ctypes.c_int64),
        ctypes.c_size_t,
    ]
    lib.axon_start_nrt_profile.restype = ctypes.c_int64
    lib.axon_stop_nrt_profile.argtypes = [ctypes.c_char_p]
    lib.axon_stop_nrt_profile.restype = ctypes.c_int64

    @contextlib.contextmanager
    def _hook(output_dir, device_ids):
        import jax

        jax.devices()
        if device_ids:
            ids = (ctypes.c_int64 * len(device_ids))(*device_ids)
            rc = lib.axon_start_nrt_profile(ids, len(device_ids))
        else:
            rc = lib.axon_start_nrt_profile(None, 0)
        if rc != 0:
            raise RuntimeError(f"axon_start_nrt_profile rc={rc}")
        try:
            yield
        finally:
            n = lib.axon_stop_nrt_profile(str(output_dir).encode())
            print(f"profile: {n} file(s) written to {output_dir}")

    import antenv

    mod = types.ModuleType("antenv.axon_hooks")
    holder = {"h": _hook}
    mod.get_axon_ntff_profile_hook = lambda: holder["h"]
    mod.set_axon_ntff_profile_hook = lambda h: holder.__setitem__("h", h)
    antenv.axon_hooks = mod
    sys.modules["antenv.axon_hooks"] = mod


_CACHED = None


def _prep_weights(inputs):
    """Fold BN into conv weights; pre-transpose/chunk projection weights."""
    f16 = np.float16
    wdiag = np.zeros((128, 81, 128), dtype=f16)
    biases = np.zeros((128, 15), dtype=np.float32)
    wproj = np.zeros((128, 18, 128), dtype=f16)
    wpv = np.zeros((128, 3, 384), dtype=f16)
    for ci, p in enumerate(["q", "k", "v"]):
        gamma = np.asarray(inputs[f"bn_{p}_gamma"], np.float64)
        var = np.asarray(inputs[f"bn_{p}_var"], np.float64)
        beta = np.asarray(inputs[f"bn_{p}_beta"], np.float64)
        mean = np.asarray(inputs[f"bn_{p}_mean"], np.float64)
        inv = gamma / np.sqrt(var + EPS)
        wfold = np.asarray(inputs[f"conv_{p}_w"], np.float64)[:, 0] * inv[:, None, None]
        bias_c = beta - mean * inv
        for tap in range(9):
            di, dj = tap // 3, tap % 3
            for cc in range(3):
                d = wfold[cc * 128 : (cc + 1) * 128, di, dj]
                np.fill_diagonal(wdiag[:, ci * 27 + tap * 3 + cc, :], d.astype(f16))
        for cc in range(3):
            biases[:, ci * 3 + cc] = bias_c[cc * 128 : (cc + 1) * 128]
        w = np.asarray(inputs[f"w_{p}"], np.float64)  # [o, c]
        assert np.abs(np.asarray(inputs[f"b_{p}"])).max() == 0.0 or p != "v", (
            "nonzero v bias unsupported"
        )
        if p == "q":
            wt = (w.T * (C**-0.5)).astype(f16)  # fold attention scale
        else:
            wt = w.T.astype(f16)
        if p in ("q", "k"):
            pi = 0 if p == "q" else 1
            for kc in range(3):
                for mc in range(3):
                    wproj[:, pi * 9 + kc * 3 + mc, :] = wt[
                        kc * 128 : (kc + 1) * 128, mc * 128 : (mc + 1) * 128
                    ]
            # projection bias (spec: zeros, but supported per out-channel)
            bvec = np.asarray(inputs[f"b_{p}"], np.float64) * (
                (C**-0.5) if p == "q" else 1.0
            )
            for mc in range(3):
                biases[:, 9 + pi * 3 + mc] = bvec[mc * 128 : (mc + 1) * 128]
        else:
            for kc in range(3):
                wpv[:, kc, :] = wt[kc * 128 : (kc + 1) * 128, :]
    return wdiag, wproj, wpv, biases


def kernel(**inputs) -> np.ndarray:
    global _CACHED, LAST_EXEC_NS
    from concourse.bass_utils import run_bass_kernel_spmd

    if TRACE:
        _install_trace_support()
    hidden = np.ascontiguousarray(np.asarray(inputs["hidden_state"], np.float32))
    assert hidden.shape == (B, L, C)
    wdiag, wproj, wpv, biases = _prep_weights(inputs)

    if _CACHED is None:
        _CACHED = _build_kernel()
    nc = _CACHED

    in_maps = []
    for core in range(NCORES):
        in_maps.append(
            {
                "hid": hidden[core * BPC : (core + 1) * BPC],
                "wdiag": wdiag,
                "wproj": wproj,
                "wpv": wpv,
                "biases": biases,
            }
        )
    res = run_bass_kernel_spmd(
        nc, in_maps, core_ids=list(range(NCORES)), trace=TRACE
    )
    LAST_EXEC_NS = res.exec_time_ns
    out = np.concatenate([res.results[i]["out"] for i in range(NCORES)], axis=0)
    return out.astype(np.float32)


# revision 25
# speedup vs baseline: 4.0424x; 1.0001x over previous
"""CvT self-attention (depthwise-conv QKV projection + MHA) on 8 Trainium2 cores.

Sharding: data-parallel over batch B=64 -> 8 batches per core. No collectives.

Per-core pipeline (per batch, all matmuls fp16 w/ fp32 PSUM accumulation):
  1. DMA hidden [1025, 384] fp32, convert fp16, PE-transpose to channel-major
     x_pad [c, 34, 34] (zero-padded spatially).
  2. Depthwise 3x3 conv + folded BN as 9 diagonal-matmul taps accumulating in
     PSUM (q: stride 1, k/v: stride 2 via strided access patterns).
  3. QKV linear projections. q/k produce [c_out, tokens]; v is computed with
     conv output as the stationary operand producing token-major [t, c_out].
  4. Attention per head, scores TRANSPOSED ([t, l]) so no transpose is needed
     between softmax and PV: scoresT = kh^T qh, exp (no max subtraction --
     scores are O(1)), PV with ones-augmented V so the softmax denominator
     falls out of the same matmul, then PE-transpose [65, l] -> [l, 65] and
     normalize by the denominator column.
"""

import sys

sys.path.insert(0, "/opt/trn_rl_repo")

import numpy as np

import concourse.bass as bass
import concourse.mybir as mybir
import concourse.tile as tile
from concourse.masks import make_identity
from concourse.vector_clock import ScopedClock

B, C, H, W = 64, 384, 32, 32
NH, HD = 6, 64
L = 1 + H * W  # 1025 query tokens
TK = 1 + (H // 2) * (W // 2)  # 257 key/value tokens
NCORES = 8
BPC = B // NCORES  # batches per core
EPS = 1e-5
F16 = mybir.dt.float16
F32 = mybir.dt.float32
Act = mybir.ActivationFunctionType

TRACE = False
LAST_EXEC_NS = None

# l chunks for the 1025-token free dim (balanced, PSUM bank = 512 fp32)
LCH = [(0, 342), (342, 342), (684, 341)]
# t chunks for the 257-token key dim over partitions
TCH = [(0, 128), (128, 128), (256, 1)]


def _split_multi_waits(nc):
    """walrus in this image only allows ONE sync wait per instruction. Move
    extra waits onto NoOps (same engine) inserted just before the offender."""
    from bass_rust import InstNoOp

    n_split = 0
    for blk in nc.m.functions[0].blocks:
        insts = blk.instructions
        out_list = []
        changed = False
        for inst in insts:
            si = inst.sync_info
            waits = list(si.on_wait) if si and si.on_wait else []
            if len(waits) > 1:
                changed = True
                for w in waits[:-1]:
                    n_split += 1
                    nop = InstNoOp(name=f"I-waitsplit-{n_split}", ins=[], outs=[])
                    nop.engine = inst.engine
                    nop.sync_info = mybir.SyncInfo(on_wait=[w], on_update=[])
                    out_list.append(nop)
                si.on_wait = waits[-1:]
            out_list.append(inst)
        if changed:
            blk.instructions = out_list


def _refuse_ldweights(nc):
    """Tile's legalizer pre-splits every matmul into LDWEIGHTS + MATMUL, but
    the InstMatmult still carries the weights AP. Drop all explicit LDWs
    (moving their waits to the next PE instruction) and let walrus --
    with --enable-ldw-opt=true -- manage weight loads itself (dedup +
    background-buffer overlap)."""
    removed = 0
    for blk in nc.m.functions[0].blocks:
        insts = blk.instructions
        out_list = []
        pending = []
        changed = False
        for inst in insts:
            if type(inst).__name__ == "InstLdweights":
                si = inst.sync_info
                if si and si.on_wait:
                    pending.extend(list(si.on_wait))
                removed += 1
                changed = True
                continue
            if pending and inst.engine == mybir.EngineType.PE:
                si = inst.sync_info
                if si is None:
                    inst.sync_info = mybir.SyncInfo(on_wait=pending, on_update=[])
                else:
                    si.on_wait = list(si.on_wait or []) + pending
                pending = []
            out_list.append(inst)
        if changed:
            blk.instructions = out_list
    return removed


def _patch_ldw_opt():
    """Let walrus dedup/overlap LDWEIGHTS (requires self-loading matmuls)."""
    import concourse.bass_utils as bu

    if getattr(bu, "_ldw_patched", False):
        return
    orig = bu.run_command

    def run_command_ldw(argv, **kw):
        argv = [
            "--enable-ldw-opt=true" if a == "--enable-ldw-opt=false" else a
            for a in argv
        ]
        return orig(argv, **kw)

    bu.run_command = run_command_ldw
    bu._ldw_patched = True


def _pair_ldw_hoist(nc):
    """Reorder [LDW_a MM_a LDW_b MM_b] -> [LDW_a LDW_b MM_a MM_b] when the two
    LDWs target disjoint PE row-groups (K<=64 at row 0 vs row 64). The PE then
    runs both matmuls concurrently (row tiling) instead of serially."""
    n = 0
    for blk in nc.m.functions[0].blocks:
        insts = blk.instructions
        changed = False
        i = 0
        while i + 3 < len(insts):
            a, b, c, e = insts[i : i + 4]
            if (
                type(a).__name__ == "InstLdweights"
                and type(b).__name__ == "InstMatmult"
                and type(c).__name__ == "InstLdweights"
                and type(e).__name__ == "InstMatmult"
                and a.tile_position is not None
                and c.tile_position is not None
                and a.tile_size is not None
                and c.tile_size is not None
                and a.tile_size[0] <= 64
                and c.tile_size[0] <= 64
                and a.tile_position[0] != c.tile_position[0]
            ):
                insts[i + 1], insts[i + 2] = insts[i + 2], insts[i + 1]
                changed = True
                n += 1
                i += 4
            else:
                i += 1
        if changed:
            blk.instructions = insts
    return n


def _patch_drain():
    """Append wait-splitting to the end of TileContext's tail drain."""
    if getattr(tile.TileContext, "_drain_patched", False):
        return

    def _drain_and_barrier(self, tick_clock, wait_clock):
        nc = self.nc
        drain_inst = nc.sync.drain()
        wait_clock.add_sem_waits(
            drain_inst.ins, ScopedClock({None: tick_clock.global_clock})
        )
        nc.all_engine_barrier()
        assert self.sems is not None
        popped = nc._tile_sem_poison_stack.pop()
        assert popped is self._sem_poison
        nc.clear_and_free_semaphores(list(self.sems.allocated().values()))
        nc.all_engine_barrier()
        _pair_ldw_hoist(nc)
        _split_multi_waits(nc)

    tile.TileContext._drain_and_barrier = _drain_and_barrier
    tile.TileContext._drain_patched = True


def _build_kernel():
    _patch_drain()
    nc = bass.Bass()
    hid = nc.dram_tensor("hid", [BPC, L, C], F32, kind="ExternalInput").ap()
    wdiag = nc.dram_tensor("wdiag", [128, 81, 128], F16, kind="ExternalInput").ap()
    wproj = nc.dram_tensor("wproj", [128, 18, 128], F16, kind="ExternalInput").ap()
    wpv = nc.dram_tensor("wpv", [128, 3, 384], F16, kind="ExternalInput").ap()
    biases = nc.dram_tensor("biases", [128, 15], F32, kind="ExternalInput").ap()
    out = nc.dram_tensor("out", [BPC, L, C], F32, kind="ExternalOutput").ap()

    with tile.TileContext(nc) as tc:
        with (
            tc.tile_pool(name="const", bufs=1) as const,
            tc.tile_pool(name="io", bufs=3) as io,
            tc.tile_pool(name="stage", bufs=2) as stage,
            tc.tile_pool(name="ctx", bufs=8) as ctxp,
            tc.tile_pool(name="outp", bufs=3) as outp,
            tc.tile_pool(name="small", bufs=4) as small,
            tc.tile_pool(name="pmm", bufs=4, space="PSUM") as pmm,
            tc.tile_pool(name="pctx", bufs=2, space="PSUM") as pctx,
            tc.tile_pool(name="ptp", bufs=2, space="PSUM") as ptp,
        ):
            # ---- constants ----
            wd_sb = const.tile([128, 81, 128], F16, tag="wd")
            nc.sync.dma_start(out=wd_sb[:], in_=wdiag)
            wp_sb = const.tile([128, 18, 128], F16, tag="wp")
            nc.sync.dma_start(out=wp_sb[:], in_=wproj)
            wpv_sb = const.tile([128, 3, 384], F16, tag="wpv")
            nc.sync.dma_start(out=wpv_sb[:], in_=wpv)
            bias_sb = const.tile([128, 15], F32, tag="bias")
            nc.sync.dma_start(out=bias_sb[:], in_=biases)
            ident = const.tile([128, 128], F16, tag="ident")
            make_identity(nc, ident[:])

            def emit_load(b):
                # ---- stage A: load (casting DMA) + PE-transpose to channel-major ----
                x_pad = stage.tile([128, 3, 34, 34], F16, tag="xpad", name=f"xpad{b}")
                # zero the 1-px border (interior is fully overwritten)
                nc.gpsimd.memset(x_pad[:, :, 0, :], 0.0)
                nc.gpsimd.memset(x_pad[:, :, 33, :], 0.0)
                nc.gpsimd.memset(x_pad[:, :, 1:33, 0], 0.0)
                nc.gpsimd.memset(x_pad[:, :, 1:33, 33], 0.0)

                for k in range(8):
                    x16 = io.tile([128, 384], F16, tag="x16", name=f"x16_{b}_{k}")
                    nc.gpsimd.dma_start(
                        out=x16[:], in_=hid[b, 1 + 128 * k : 1 + 128 * (k + 1), :]
                    )
                    for cc in range(3):
                        tp = ptp.tile([128, 128], F16, tag="tp", name=f"tpi{b}_{k}_{cc}")
                        nc.tensor.transpose(
                            tp[:], x16[:, cc * 128 : (cc + 1) * 128], ident[:]
                        )
                        # tokens 128k..128k+127 = image rows 4k..4k+3
                        nc.vector.tensor_copy(
                            x_pad[:, cc, 1 + 4 * k : 5 + 4 * k, 1:33], tp[:]
                        )

                cls16 = small.tile([128, 3], F16, tag="cls", name=f"cls{b}")
                for cc in range(3):
                    nc.gpsimd.dma_start(
                        out=cls16[:, cc : cc + 1],
                        in_=hid[b, 0:1, cc * 128 : (cc + 1) * 128].rearrange(
                            "a b -> b a"
                        ),
                    )
                return {"x_pad": x_pad, "cls16": cls16}

            def emit_convproj(b, st):
                x_pad, cls16 = st["x_pad"], st["cls16"]
                # ---- stage B: depthwise conv + BN (diagonal matmuls) ----
                q_src = stage.tile([128, 3, 1025], F16, tag="qsrc", name=f"qsrc{b}")
                k_src = stage.tile([128, 3, 257], F16, tag="ksrc", name=f"ksrc{b}")
                v_src = stage.tile([128, 3, 257], F16, tag="vsrc", name=f"vsrc{b}")
                for cc in range(3):
                    for s in (q_src, k_src, v_src):
                        nc.gpsimd.tensor_copy(s[:, cc, 0:1], cls16[:, cc : cc + 1])
                for cc in range(3):
                    # q: stride 1, two 512-token banks; taps outer so both
                    # banks' matmuls share one LDWEIGHTS
                    psq = [
                        pmm.tile([128, 512], F32, tag="mm", name=f"psq{b}_{cc}_{i}")
                        for i in range(2)
                    ]
                    for tap in range(9):
                        di, dj = tap // 3, tap % 3
                        for nb in range(2):
                            rhs = x_pad[
                                :, cc, 16 * nb + di : 16 * nb + di + 16, dj : dj + 32
                            ]
                            nc.tensor.matmul(
                                psq[nb][:],
                                wd_sb[:, tap * 3 + cc, :],
                                rhs,
                                start=(tap == 0),
                                stop=(tap == 8),
                            )
                    for nb in range(2):
                        nc.scalar.activation(
                            q_src[:, cc, 1 + 512 * nb : 513 + 512 * nb],
                            psq[nb][:],
                            Act.Identity,
                            bias=bias_sb[:, cc : cc + 1],
                        )
                    # k, v: stride 2 (16x16 outputs)
                    xv = x_pad[:, cc].rearrange(
                        "p (i ti) (j tj) -> p i ti j tj", ti=2, tj=2
                    )
                    for ci, src in ((1, k_src), (2, v_src)):
                        ps = pmm.tile([128, 512], F32, tag="mm", name=f"pkv{b}_{cc}_{ci}")
                        for tap in range(9):
                            di, dj = tap // 3, tap % 3
                            rhs = xv[
                                :,
                                di // 2 : di // 2 + 16,
                                di % 2,
                                dj // 2 : dj // 2 + 16,
                                dj % 2,
                            ]
                            nc.tensor.matmul(
                                ps[:, :256],
                                wd_sb[:, ci * 27 + tap * 3 + cc, :],
                                rhs,
                                start=(tap == 0),
                                stop=(tap == 8),
                            )
                        nc.scalar.activation(
                            src[:, cc, 1:257],
                            ps[:, :256],
                            Act.Identity,
                            bias=bias_sb[:, ci * 3 + cc : ci * 3 + cc + 1],
                        )

                # ---- stage C: projections (kc inner-adjacent for LDW reuse) ----
                qh = stage.tile([128, 3, 1025], F16, tag="qh", name=f"qh{b}")
                kh = stage.tile([128, 3, 257], F16, tag="kh", name=f"kh{b}")
                for mc in range(3):
                    ps3 = [
                        pmm.tile([128, 512], F32, tag="mm", name=f"ps3_{b}_{mc}_{i}")
                        for i in range(3)
                    ]
                    for kc in range(3):
                        for nci, (n0, nl) in enumerate(LCH):
                            nc.tensor.matmul(
                                ps3[nci][:, :nl],
                                wp_sb[:, kc * 3 + mc, :],
                                q_src[:, kc, n0 : n0 + nl],
                                start=(kc == 0),
                                stop=(kc == 2),
                            )
                    for nci, (n0, nl) in enumerate(LCH):
                        nc.vector.tensor_scalar_add(
                            qh[:, mc, n0 : n0 + nl],
                            ps3[nci][:, :nl],
                            bias_sb[:, 9 + mc : 10 + mc],
                        )
                    ps = pmm.tile([128, 512], F32, tag="mm", name=f"pk{b}_{mc}")
                    for kc in range(3):
                        nc.tensor.matmul(
                            ps[:, :257],
                            wp_sb[:, 9 + kc * 3 + mc, :],
                            k_src[:, kc, :],
                            start=(kc == 0),
                            stop=(kc == 2),
                        )
                    nc.vector.tensor_scalar_add(
                        kh[:, mc, :], ps[:, :257], bias_sb[:, 12 + mc : 13 + mc]
                    )
                v_store = stage.tile([128, 3, 6, 65], F16, tag="vst", name=f"vst{b}")
                nc.gpsimd.memset(v_store[:, :, :, 64:65], 1.0)
                for tcc, (t0, tl) in enumerate(TCH):
                    ps = pmm.tile([128, 512], F32, tag="mm", name=f"pv{b}_{tcc}")
                    for kc in range(3):
                        nc.tensor.matmul(
                            ps[:tl, :384],
                            v_src[:, kc, t0 : t0 + tl],
                            wpv_sb[:, kc, :],
                            start=(kc == 0),
                            stop=(kc == 2),
                        )
                    nc.vector.tensor_copy(
                        v_store[:tl, tcc, :, 0:64],
                        ps[:tl, :384].rearrange("p (h d) -> p h d", h=6),
                    )
                st.update(qh=qh, kh=kh, v_store=v_store)

            def emit_attn(b, st):
                qh, kh, v_store = st["qh"], st["kh"], st["v_store"]
                # ---- stage D/E: attention, heads in pairs. The pair lives
                # at PE row-groups 0-63 / 64-127, so its score matmuls run
                # CONCURRENTLY on the array (row tiling) when interleaved.
                ctx_tiles = [None] * 6
                for hp in range(3):
                    h0, h1 = 2 * hp, 2 * hp + 1
                    ch = hp
                    exps = [
                        stage.tile(
                            [128, 3, 1025], F16, tag="expT", bufs=4,
                            name=f"expT{b}_{h}",
                        )
                        for h in (h0, h1)
                    ]
                    for tcc, (t0, tl) in enumerate(TCH):
                        for n0, nl in LCH:
                            pspair = [
                                pmm.tile(
                                    [128, 512], F32, tag="mm",
                                    name=f"sc{b}_{h}_{tcc}_{n0}",
                                )
                                for h in (h0, h1)
                            ]
                            for side in range(2):
                                base = side * 64
                                nc.tensor.matmul(
                                    pspair[side][:tl, :nl],
                                    kh[base : base + 64, ch, t0 : t0 + tl],
                                    qh[base : base + 64, ch, n0 : n0 + nl],
                                    start=True,
                                    stop=True,
                                )
                            for side in range(2):
                                nc.scalar.activation(
                                    exps[side][:tl, tcc, n0 : n0 + nl],
                                    pspair[side][:tl, :nl],
                                    Act.Exp,
                                )
                    # PV: tc outer so the 3 l-chunks share each LDWEIGHTS
                    for side, h in ((0, h0), (1, h1)):
                        ctxsb = ctxp.tile(
                            [128, 1152], F16, tag="ctx", name=f"ctx{b}_{h}"
                        )
                        ctx_tiles[h] = ctxsb
                        for nci, (n0, nl) in enumerate(LCH):
                            cps = pctx.tile(
                                [65, 512], F32, tag="pc", name=f"cps{b}_{h}_{nci}"
                            )
                            for tcc, (t0, tl) in enumerate(TCH):
                                nc.tensor.matmul(
                                    cps[:, :nl],
                                    v_store[:tl, tcc, h, :],
                                    exps[side][:tl, tcc, n0 : n0 + nl],
                                    start=(tcc == 0),
                                    stop=(tcc == 2),
                                )
                            nc.scalar.activation(
                                ctxsb[:65, n0 : n0 + nl], cps[:, :nl], Act.Copy
                            )

                # ---- stage F: PE-transpose back, normalize, store ----
                for lc in range(9):
                    l0 = lc * 128
                    ll = min(128, L - l0)
                    osb = outp.tile([128, 384], F32, tag="osb", name=f"osb{b}_{lc}")
                    for h in range(6):
                        tp = ptp.tile([128, 65], F16, tag="tp", name=f"tpc{b}_{lc}_{h}")
                        nc.tensor.transpose(
                            tp[:ll, :],
                            ctx_tiles[h][:65, l0 : l0 + ll],
                            ident[:65, :65],
                        )
                        rec = small.tile([128, 1], F32, tag="rec", name=f"rec{b}_{lc}_{h}")
                        nc.vector.reciprocal(rec[:ll], tp[:ll, 64:65])
                        nc.vector.tensor_scalar_mul(
                            osb[:ll, h * 64 : (h + 1) * 64], tp[:ll, 0:64], rec[:ll]
                        )
                    nc.sync.dma_start(
                        out=out[b, l0 : l0 + ll, :], in_=osb[:ll, :]
                    )

            # software pipeline: batch b's conv/proj is emitted before batch
            # b-1's attention-dependent work drains, giving the PE dense
            # filler while ACT computes the exps.
            state = {}
            for b in range(BPC):
                state[b] = emit_load(b)
                if b >= 1:
                    emit_attn(b - 1, state.pop(b - 1))
                emit_convproj(b, state[b])
            emit_attn(BPC - 1, state.pop(BPC - 1))
    return nc


def _install_trace_support():
    """Provide the NTFF profile hook (this image's antenv lacks axon_hooks)
    and neuter the artifact upload (no fish access here)."""
    import contextlib
    import ctypes
    import types

    import concourse.bass_utils as bu

    bu.upload_artifacts = lambda tmpdir: f"local:{tmpdir}"
    try:
        from antenv.axon_hooks import get_axon_ntff_profile_hook  # noqa: F401

        return
    except ImportError:
        pass
    so_path = "/opt/axon/libaxon_pjrt.so"
    lib = ctypes.CDLL(so_path)
    if not hasattr(lib, "axon_start_nrt_profile"):
        return
    lib.axon_start_nrt_profile.argtypes = [
        ctypes.POINTER(ctypes.c_int64),
        ctypes.c_size_t,
    ]
    lib.axon_start_nrt_profile.restype = ctypes.c_int64
    lib.axon_stop_nrt_profile.argtypes = [ctypes.c_char_p]
    lib.axon_stop_nrt_profile.restype = ctypes.c_int64

    @contextlib.contextmanager
    def _hook(output_dir, device_ids):
        import jax

        jax.devices()
        if device_ids:
            ids = (ctypes.c_int64 * len(device_ids))(*device_ids)
            rc = lib.axon_start_nrt_profile(ids, len(device_ids))
        else:
            rc = lib.axon_start_nrt_profile(None, 0)
        if rc != 0:
            raise RuntimeError(f"axon_start_nrt_profile rc={rc}")
        try:
            yield
        finally:
            n = lib.axon_stop_nrt_profile(str(output_dir).encode())
            print(f"profile: {n} file(s) written to {output_dir}")

    import antenv

    mod = types.ModuleType("antenv.axon_hooks")
    holder = {"h": _hook}
    mod.get_axon_ntff_profile_hook = lambda: holder["h"]
    mod.set_axon_ntff_profile_hook = lambda h: holder.__setitem__("h", h)
    antenv.axon_hooks = mod
    sys.modules["antenv.axon_hooks"] = mod


_CACHED = None


def _prep_weights(inputs):
    """Fold BN into conv weights; pre-transpose/chunk projection weights."""
    f16 = np.float16
    wdiag = np.zeros((128, 81, 128), dtype=f16)
    biases = np.zeros((128, 15), dtype=np.float32)
    wproj = np.zeros((128, 18, 128), dtype=f16)
    wpv = np.zeros((128, 3, 384), dtype=f16)
    for ci, p in enumerate(["q", "k", "v"]):
        gamma = np.asarray(inputs[f"bn_{p}_gamma"], np.float64)
        var = np.asarray(inputs[f"bn_{p}_var"], np.float64)
        beta = np.asarray(inputs[f"bn_{p}_beta"], np.float64)
        mean = np.asarray(inputs[f"bn_{p}_mean"], np.float64)
        inv = gamma / np.sqrt(var + EPS)
        wfold = np.asarray(inputs[f"conv_{p}_w"], np.float64)[:, 0] * inv[:, None, None]
        bias_c = beta - mean * inv
        for tap in range(9):
            di, dj = tap // 3, tap % 3
            for cc in range(3):
                d = wfold[cc * 128 : (cc + 1) * 128, di, dj]
                np.fill_diagonal(wdiag[:, ci * 27 + tap * 3 + cc, :], d.astype(f16))
        for cc in range(3):
            biases[:, ci * 3 + cc] = bias_c[cc * 128 : (cc + 1) * 128]
        w = np.asarray(inputs[f"w_{p}"], np.float64)  # [o, c]
        assert np.abs(np.asarray(inputs[f"b_{p}"])).max() == 0.0 or p != "v", (
            "nonzero v bias unsupported"
        )
        if p == "q":
            wt = (w.T * (C**-0.5)).astype(f16)  # fold attention scale
        else:
            wt = w.T.astype(f16)
        if p in ("q", "k"):
            pi = 0 if p == "q" else 1
            for kc in range(3):
                for mc in range(3):
                    wproj[:, pi * 9 + kc * 3 + mc, :] = wt[
                        kc * 128 : (kc + 1) * 128, mc * 128 : (mc + 1) * 128
                    ]
            # projection bias (spec: zeros, but supported per out-channel)
            bvec = np.asarray(inputs[f"b_{p}"], np.float64) * (
                (C**-0.5) if p == "q" else 1.0
            )
            for mc in range(3):
                biases[:, 9 + pi * 3 + mc] = bvec[mc * 128 : (mc + 1) * 128]
        else:
            for kc in range(3):
                wpv[:, kc, :] = wt[kc * 128 : (kc + 1) * 128, :]
    return wdiag, wproj, wpv, biases


def kernel(**inputs) -> np.ndarray:
    global _CACHED, LAST_EXEC_NS
    from concourse.bass_utils import run_bass_kernel_spmd

    if TRACE:
        _install_trace_support()
    hidden = np.ascontiguousarray(np.asarray(inputs["hidden_state"], np.float32))
    assert hidden.shape == (B, L, C)
    wdiag, wproj, wpv, biases = _prep_weights(inputs)

    if _CACHED is None:
        _CACHED = _build_kernel()
    nc = _CACHED

    in_maps = []
    for core in range(NCORES):
        in_maps.append(
            {
                "hid": hidden[core * BPC : (core + 1) * BPC],
                "wdiag": wdiag,
                "wproj": wproj,
                "wpv": wpv,
                "biases": biases,
            }
        )
    res = run_bass_kernel_spmd(
        nc, in_maps, core_ids=list(range(NCORES)), trace=TRACE
    )
    LAST_EXEC_NS = res.exec_time_ns
    out = np.concatenate([res.results[i]["out"] for i in range(NCORES)], axis=0)
    return out.astype(np.float32)
